# revision 5
# baseline (speedup 1.0000x reference)
"""BiLSTM-CRF forward loss on 8 Trainium2 NeuronCores.

Data-parallel: batch 64 -> 8 sequences per core. Each core runs
embedding gather -> BiLSTM(T=512,H=256) -> fc1(32)+relu -> fc2(4),
and outputs its emissions [4, T*8]. The tiny CRF dynamic program
(O(T*B*K^2), K=4) and the final mean run on host in numpy.

v2: FP8 DoubleRow matmuls (256-deep contraction per instruction) for
the recurrent, input and fc1 weights; the xg add is folded into PSUM
accumulation via an fp8 identity matmul; tanh(g) is computed as
2*sigmoid(2g)-1 with the 2x folded into the weights and the affine
fixup fused into the DVE c-update, so one sigmoid activation covers
all 64 gate columns; the h-gate multiply runs on the Pool engine.
"""

import sys
for _p in ("/opt/trn_rl_repo", "/root/.axon_site/_ro/trn_rl_repo"):
    if _p not in sys.path:
        sys.path.insert(0, _p)
import numpy as np
from ml_dtypes import bfloat16

import concourse.bass as bass
import concourse.bacc as bacc
import concourse.mybir as mybir
from concourse.tile import TileContext
from concourse import bass_utils

B, T, E, H, V, K = 64, 512, 300, 256, 50000, 4
NCORES = 8
BC = B // NCORES          # 8 sequences per core
EP = 304                  # E padded to 304; row 300 = ones (bias trick)
G4H = 4 * H               # 1024
F32 = mybir.dt.float32
BF16 = mybir.dt.bfloat16
I32 = mybir.dt.int32
FP8 = mybir.dt.float8e4
AF = mybir.ActivationFunctionType
ALU = mybir.AluOpType
DR = mybir.MatmulPerfMode.DoubleRow


def build_bass(t_steps=T, bc=BC, parts="012f"):
    TOK = t_steps * bc
    nc = bacc.Bacc()

    # ---- DRAM parameters ----
    emb_aug = nc.dram_tensor("emb_aug", [V, EP], BF16, kind="ExternalInput")
    toks = nc.dram_tensor("toks", [TOK, 1], I32, kind="ExternalInput")
    # DoubleRow-packed input weights: rows 0:256 -> [128, 2*G4H], tail 48
    wxf = nc.dram_tensor("wxf", [128, 2 * G4H], FP8, kind="ExternalInput")
    wxb = nc.dram_tensor("wxb", [128, 2 * G4H], FP8, kind="ExternalInput")
    wxf_t = nc.dram_tensor("wxf_t", [48, G4H], FP8, kind="ExternalInput")
    wxb_t = nc.dram_tensor("wxb_t", [48, G4H], FP8, kind="ExternalInput")
    whf = nc.dram_tensor("whf", [128, 2 * G4H], FP8, kind="ExternalInput")
    whb = nc.dram_tensor("whb", [128, 2 * G4H], FP8, kind="ExternalInput")
    fc1w = nc.dram_tensor("fc1w", [128, 4 * 32], FP8, kind="ExternalInput")
    fc1b = nc.dram_tensor("fc1b", [32, 1], F32, kind="ExternalInput")
    fc2w = nc.dram_tensor("fc2w", [32, K], BF16, kind="ExternalInput")
    fc2b = nc.dram_tensor("fc2b", [K, 1], F32, kind="ExternalInput")
    iden = nc.dram_tensor("iden", [128, 128], BF16, kind="ExternalInput")
    iden8 = nc.dram_tensor("iden8", [128, 128], FP8, kind="ExternalInput")
    out = nc.dram_tensor("out", [K, TOK], F32, kind="ExternalOutput")

    n_ttile = TOK // 128          # token tiles of 128
    n_n512 = TOK // 512           # 512-wide token chunks
    ek = [(0, 128), (128, 128), (256, 48)]   # E-chunks for transpose

    with TileContext(nc) as tc:
        with tc.tile_pool(name="const", bufs=1) as constp, \
             tc.tile_pool(name="persist", bufs=1) as pp:
            # constants in SBUF
            id_sb = constp.tile([128, 128], BF16, tag="iden")
            nc.sync.dma_start(id_sb[:], iden[:])
            id8_sb = constp.tile([128, 128], FP8, tag="iden8")
            nc.sync.dma_start(id8_sb[:], iden8[:])
            wx_sb, wxt_sb, wh_sb = {}, {}, {}
            for d, src, srct, srch in (("f", wxf, wxf_t, whf),
                                       ("b", wxb, wxb_t, whb)):
                w = constp.tile([128, 2 * G4H], FP8, tag=f"wx{d}")
                nc.sync.dma_start(w[:], src[:])
                wx_sb[d] = w
                w = constp.tile([48, G4H], FP8, tag=f"wxt{d}")
                nc.sync.dma_start(w[:], srct[:])
                wxt_sb[d] = w
                w = constp.tile([128, 2 * G4H], FP8, tag=f"wh{d}")
                nc.sync.dma_start(w[:], srch[:])
                wh_sb[d] = w
            fc1w_sb = constp.tile([128, 4 * 32], FP8, tag="fc1w")
            nc.sync.dma_start(fc1w_sb[:], fc1w[:])
            fc2w_sb = constp.tile([32, K], BF16, tag="fc2w")
            nc.sync.dma_start(fc2w_sb[:], fc2w[:])
            fc1b_sb = constp.tile([32, 1], F32, tag="fc1b")
            nc.sync.dma_start(fc1b_sb[:], fc1b[:])
            fc2b_sb = constp.tile([K, 1], F32, tag="fc2b")
            nc.sync.dma_start(fc2b_sb[:], fc2b[:])

            # persistent activations
            # xg layout: [128, 8 gate-chunks * TOK], col = mu*TOK + t*bc + b
            # gate chunk order (host permutes): i,i,f,f,o,o,g,g
            xg = {d: pp.tile([128, 8 * TOK], FP8, tag=f"xg{d}", name=f"xg{d}")
                  for d in "fb"}
            # h layout: [128, 2 hid-chunks * TOK] fp8, col = k*TOK + t*bc + b
            hT = {d: pp.tile([128, 2 * TOK], FP8, tag=f"h{d}", name=f"h{d}")
                  for d in "fb"}

            with tc.tile_pool(name="xt", bufs=1) as xtp, \
                 tc.tile_pool(name="xrp", bufs=2) as xrp, \
                 tc.tile_pool(name="rec", bufs=1) as recp, \
                 tc.tile_pool(name="st", bufs=3) as stp, \
                 tc.tile_pool(name="fc", bufs=1) as fcp, \
                 tc.tile_pool(name="emo", bufs=2) as emop, \
                 tc.tile_pool(name="ps0", bufs=2, space="PSUM") as ps0p, \
                 tc.tile_pool(name="pst", bufs=2, space="PSUM") as pstp, \
                 tc.tile_pool(name="ps2", bufs=2, space="PSUM") as ps2p:
                # ---------- phase 0: gather + transpose -> xT (fp8) --------
                # xT01: k-tile pair (rows 0:128 | 128:256) for DoubleRow rhs
                xT01 = xtp.tile([128, 2 * TOK], FP8, tag="xT01", name="xT01")
                xT2 = xtp.tile([48, TOK], FP8, tag="xT2", name="xT2")
                idx_all = xtp.tile([128, n_ttile], I32, tag="idx_all")
                nc.gpsimd.dma_start(
                    idx_all[:],
                    toks[:].rearrange("(i p) one -> p (i one)", p=128))
                GC = min(8, n_ttile)        # token tiles per gather chunk
                if "0" not in parts:
                    GC = 0
                for c0 in range(0, n_ttile, GC if GC else n_ttile + 1):
                    xr = xrp.tile([128, GC * EP], BF16, tag="xr", name="xr")
                    for j in range(GC):
                        i = c0 + j
                        nc.gpsimd.indirect_dma_start(
                            out=xr[:, j * EP:(j + 1) * EP], out_offset=None,
                            in_=emb_aug[:],
                            in_offset=bass.IndirectOffsetOnAxis(
                                ap=idx_all[:, i:i + 1], axis=0),
                        )
                    for j in range(GC):
                        i = c0 + j
                        for ki, (r0, rn) in enumerate(ek):
                            pt = pstp.tile([128, 128], BF16, tag="tp")
                            nc.tensor.transpose(
                                out=pt[:rn, :],
                                in_=xr[:, j * EP + r0:j * EP + r0 + rn],
                                identity=id_sb[:])
                            if ki < 2:
                                dst = xT01[:, ki * TOK + i * 128:
                                           ki * TOK + (i + 1) * 128]
                            else:
                                dst = xT2[:, i * 128:(i + 1) * 128]
                            nc.vector.tensor_copy(out=dst, in_=pt[:rn, :])

                xT01v = xT01[:].rearrange("p (k t) -> p k t", k=2)
                for d in "fb":
                    # ---- phase 1: xg = wx^T @ x (DoubleRow + 48-row tail) --
                    wxv = wx_sb[d][:].rearrange("p (k g) -> p k g", k=2)
                    for mu in range(8):
                        for n in range(n_n512):
                            ps = ps0p.tile([128, 512], F32, tag="mm")
                            nc.tensor.matmul(
                                ps[:],
                                lhsT=wxv[:, :, mu * 128:(mu + 1) * 128],
                                rhs=xT01v[:, :, n * 512:(n + 1) * 512],
                                start=True, stop=False,
                                perf_mode=DR, skip_group_check=True)
                            nc.tensor.matmul(
                                ps[:],
                                lhsT=wxt_sb[d][:, mu * 128:(mu + 1) * 128],
                                rhs=xT2[:, n * 512:(n + 1) * 512],
                                start=False, stop=True,
                                skip_group_check=True)
                            dst = xg[d][:, mu * TOK + n * 512:
                                        mu * TOK + (n + 1) * 512]
                            if n % 2 == 0:
                                nc.scalar.copy(out=dst, in_=ps[:])
                            else:
                                nc.vector.tensor_copy(out=dst, in_=ps[:])

                # -------- phase 2: recurrence, fwd+bwd interleaved ----------
                # sa cols per dir: [i(16) f(16) o(16) g2(16)] (ki-major pairs)
                c_st = {d: recp.tile([128, 2 * bc], F32, tag=f"c{d}",
                                     name=f"c{d}") for d in "fb"}
                hTv = {d: hT[d][:].rearrange("p (k t) -> p k t", k=2)
                       for d in "fb"}
                whv = {d: wh_sb[d][:].rearrange("p (k g) -> p k g", k=2)
                       for d in "fb"}
                for step in range(t_steps):
                    for d in "fb":
                        t = step if d == "f" else t_steps - 1 - step
                        first = step == 0
                        ps = ps2p.tile([128, 8 * bc], F32, tag=f"ps{d}",
                                       name=f"ps{d}")
                        xga = xg[d][:].rearrange(
                            "p (m t) -> p m t", m=8)[:, :, t * bc:(t + 1) * bc]
                        # xg add via fp8 identity matmul (starts accumulation)
                        nc.tensor.matmul(
                            ps[:], lhsT=id8_sb[:], rhs=xga,
                            start=True, stop=first, skip_group_check=True)
                        if not first:
                            tprev = t - 1 if d == "f" else t + 1
                            for mu in range(8):
                                nc.tensor.matmul(
                                    ps[:, mu * bc:(mu + 1) * bc],
                                    lhsT=whv[d][:, :, mu * 128:(mu + 1) * 128],
                                    rhs=hTv[d][:, :, tprev * bc:(tprev + 1) * bc],
                                    start=False, stop=True,
                                    perf_mode=DR, skip_group_check=True)
                        sa = stp.tile([128, 8 * bc], F32, tag=f"sa{d}",
                                      name=f"sa{d}")
                        nc.scalar.activation(sa[:], ps[:], AF.Sigmoid)
                        # c = sig_f*c + sig_i*(2*sig_g2 - 1)
                        t2 = stp.tile([128, 2 * bc], F32, tag=f"t2{d}",
                                      name=f"t2{d}")
                        nc.vector.scalar_tensor_tensor(
                            out=t2[:], in0=sa[:, 6 * bc:8 * bc], scalar=-0.5,
                            in1=sa[:, 0:2 * bc], op0=ALU.add, op1=ALU.mult)
                        if first:
                            nc.vector.tensor_scalar_mul(
                                c_st[d][:], t2[:], 2.0)
                        else:
                            # t1 on Pool runs concurrently with t2' on DVE
                            t1 = stp.tile([128, 2 * bc], F32, tag=f"t1{d}",
                                          name=f"t1{d}")
                            nc.gpsimd.tensor_tensor(
                                out=t1[:], in0=sa[:, 2 * bc:4 * bc],
                                in1=c_st[d][:], op=ALU.mult)
                            nc.vector.scalar_tensor_tensor(
                                out=c_st[d][:], in0=t2[:], scalar=2.0,
                                in1=t1[:], op0=ALU.mult, op1=ALU.add)
                        tcl = stp.tile([128, 2 * bc], F32, tag=f"tc{d}",
                                       name=f"tc{d}")
                        nc.scalar.activation(tcl[:], c_st[d][:], AF.Tanh)
                        # h = sig_o * tanh(c) on the Pool engine, fp8 out
                        nc.gpsimd.tensor_tensor(
                            out=hTv[d][:, :, t * bc:(t + 1) * bc],
                            in0=sa[:, 4 * bc:6 * bc].rearrange(
                                "p (k c) -> p k c", k=2),
                            in1=tcl[:].rearrange("p (k c) -> p k c", k=2),
                            op=ALU.mult)

                # ---------- phase 3: fc1 + relu, fc2 + bias, out ----------
                z = fcp.tile([32, TOK], BF16, tag="z")
                fc1v = fc1w_sb[:].rearrange("p (dk c) -> p dk c", dk=4)
                n_n512_f = n_n512 if "f" in parts else 0
                for n in range(n_n512_f):
                    ps = ps0p.tile([32, 512], F32, tag="mm")
                    for di, d in enumerate("fb"):
                        nc.tensor.matmul(
                            ps[:],
                            lhsT=fc1v[:, 2 * di:2 * di + 2, :],
                            rhs=hTv[d][:, :, n * 512:(n + 1) * 512],
                            start=(di == 0), stop=(di == 1),
                            perf_mode=DR, skip_group_check=True)
                    nc.scalar.activation(z[:, n * 512:(n + 1) * 512], ps[:],
                                         AF.Relu, bias=fc1b_sb[:, :1])
                for n in range(n_n512_f):
                    ps = ps0p.tile([K, 512], F32, tag="mm")
                    nc.tensor.matmul(ps[:], lhsT=fc2w_sb[:],
                                     rhs=z[:, n * 512:(n + 1) * 512],
                                     start=True, stop=True)
                    em = emop.tile([K, 512], F32, tag="em", name="em")
                    nc.vector.tensor_scalar_add(em[:], ps[:], fc2b_sb[:, :1])
                    nc.sync.dma_start(out[:, n * 512:(n + 1) * 512], em[:])
    nc.compile()
    return nc


def _pack_dr(m):
    """[256, N] -> DoubleRow-packed [128, 2*N]: out[p, k*N+j] = m[k*128+p, j]"""
    n = m.shape[1]
    return m.reshape(2, 128, n).transpose(1, 0, 2).reshape(128, 2 * n)


def _prep_shared(emb, w_ih_f, w_hh_f, b_ih_f, b_hh_f, w_ih_b, w_hh_b,
                 b_ih_b, b_hh_b, fc1_w, fc1_b, fc2_w, fc2_b):
    f32 = np.float32
    fp8 = mybir.dt.np(FP8)
    emb_aug = np.zeros((V, EP), f32)
    emb_aug[:, :E] = np.asarray(emb, f32)
    emb_aug[0, :E] = 0.0
    emb_aug[:, E] = 1.0

    perm = np.r_[0:512, 768:1024, 512:768]  # i,f,g,o -> i,f,o,g

    def wx(w_ih, b_ih, b_hh):
        m = np.zeros((EP, G4H), f32)
        m[:E, :] = np.asarray(w_ih, f32).T
        m[E, :] = np.asarray(b_ih, f32) + np.asarray(b_hh, f32)
        m = m[:, perm]
        m[:, 768:] *= 2.0          # tanh(g) = 2*sigmoid(2g) - 1
        return m

    def wh(w_hh):
        m = np.asarray(w_hh, f32).T[:, perm].copy()
        m[:, 768:] *= 2.0
        return m

    wxf_full = wx(w_ih_f, b_ih_f, b_hh_f)
    wxb_full = wx(w_ih_b, b_ih_b, b_hh_b)
    fc1 = np.asarray(fc1_w, f32).T        # [512, 32]
    fc1_pk = np.concatenate(
        [_pack_dr(fc1[0:256]), _pack_dr(fc1[256:512])], axis=1)  # [128, 4*32]

    return dict(
        emb_aug=emb_aug.astype(bfloat16).copy(),
        wxf=_pack_dr(wxf_full[0:256]).astype(fp8).copy(),
        wxb=_pack_dr(wxb_full[0:256]).astype(fp8).copy(),
        wxf_t=wxf_full[256:304].astype(fp8).copy(),
        wxb_t=wxb_full[256:304].astype(fp8).copy(),
        whf=_pack_dr(wh(w_hh_f)).astype(fp8).copy(),
        whb=_pack_dr(wh(w_hh_b)).astype(fp8).copy(),
        fc1w=fc1_pk.astype(fp8).copy(),
        fc1b=np.asarray(fc1_b, f32).reshape(32, 1).copy(),
        fc2w=np.asarray(fc2_w, f32).T.astype(bfloat16).copy(),
        fc2b=np.asarray(fc2_b, f32).reshape(K, 1).copy(),
        iden=np.eye(128, dtype=f32).astype(bfloat16).copy(),
        iden8=np.eye(128, dtype=f32).astype(fp8).copy(),
    )


def _crf_host(emis, tags, mask, start_trans, trans, end_trans):
    # emis: [T, B, K] f32; exact forward algorithm in float64 on host
    emis = emis.astype(np.float64)
    trans = np.asarray(trans, np.float64)
    start = np.asarray(start_trans, np.float64)
    end = np.asarray(end_trans, np.float64)
    tags = np.asarray(tags, np.int64)
    m = np.asarray(mask, np.float64).T           # [T, B]
    tg = tags.T                                  # [T, B]
    Bsz = emis.shape[1]
    bidx = np.arange(Bsz)

    score = start[tg[0]] + emis[0, bidx, tg[0]]
    for t in range(1, emis.shape[0]):
        score = score + (trans[tg[t - 1], tg[t]] + emis[t, bidx, tg[t]]) * m[t]
    seq_ends = np.asarray(mask, np.int64).sum(1) - 1
    score = score + end[tg[seq_ends, bidx]]

    alpha = start[None, :] + emis[0]
    for t in range(1, emis.shape[0]):
        nxt = alpha[:, :, None] + trans[None] + emis[t][:, None, :]
        mx = nxt.max(axis=1)
        nxt = mx + np.log(np.exp(nxt - mx[:, None, :]).sum(axis=1))
        alpha = np.where(m[t][:, None] > 0, nxt, alpha)
    av = alpha + end[None, :]
    mx = av.max(axis=1)
    logZ = mx + np.log(np.exp(av - mx[:, None]).sum(axis=1))
    return -(score - logZ).mean()


_CACHE = {}


def _make_runner():
    import jax
    from jax.sharding import Mesh, PartitionSpec, NamedSharding
    try:
        from jax.experimental.shard_map import shard_map
    except ImportError:
        from jax import shard_map
    from concourse import bass2jax
    from concourse.bass2jax import _bass_exec_p, partition_id_tensor

    nc = build_bass()
    bass2jax.install_neuronx_cc_hook()
    partition_name = (nc.partition_id_tensor.name
                      if nc.partition_id_tensor else None)
    in_names, out_names, out_avals, zero_outs = [], [], [], []
    for alloc in nc.m.functions[0].allocations:
        if not isinstance(alloc, mybir.MemoryLocationSet):
            continue
        name = alloc.memorylocations[0].name
        if alloc.kind == "ExternalInput":
            if name != partition_name:
                in_names.append(name)
        elif alloc.kind == "ExternalOutput":
            shape = tuple(alloc.tensor_shape)
            dtype = mybir.dt.np(alloc.dtype)
            out_names.append(name)
            out_avals.append(jax.core.ShapedArray(shape, dtype))
            zero_outs.append(np.zeros(shape, dtype))
    n_params = len(in_names)
    in_names_all = in_names + out_names
    if partition_name is not None:
        in_names_all.append(partition_name)

    def _body(*args):
        operands = list(args)
        if partition_name is not None:
            operands.append(partition_id_tensor())
        return tuple(_bass_exec_p.bind(
            *operands, out_avals=tuple(out_avals),
            in_names=tuple(in_names_all), out_names=tuple(out_names),
            lowering_input_output_aliases=(),
            sim_require_finite=True, sim_require_nnan=True, nc=nc))

    devices = jax.devices()[:NCORES]
    mesh = Mesh(np.asarray(devices), ("core",))
    sh = NamedSharding(mesh, PartitionSpec("core"))
    # The kernel writes every element of its outputs, so the zero output
    # buffers are NOT donated: they are uploaded once and reused by every
    # execution (saves one ~70ms host->device sync per run).
    sm = shard_map(_body, mesh=mesh,
                   in_specs=(PartitionSpec("core"),) * (n_params + len(out_names)),
                   out_specs=(PartitionSpec("core"),) * len(out_names),
                   check_rep=False)
    return dict(jax=jax, sm=sm, sh=sh, in_names=in_names,
                out_names=out_names, zero_outs=zero_outs)


def _run_device(in_maps):
    if "rt" not in _CACHE:
        _CACHE["rt"] = _make_runner()
    rt = _CACHE["rt"]
    jax = rt["jax"]
    from concourse.bass2jax import fast_dispatch_compile
    concat_in = [np.concatenate([np.asarray(m[n]) for m in in_maps], 0)
                 for n in rt["in_names"]]
    rt["dev_in"] = [jax.device_put(a, rt["sh"]) for a in concat_in]
    rt["zo_dev"] = [jax.device_put(np.concatenate([z] * NCORES, 0), rt["sh"])
                    for z in rt["zero_outs"]]
    if "sharded" not in rt:
        args = tuple(rt["dev_in"]) + tuple(rt["zo_dev"])
        try:
            rt["sharded"] = fast_dispatch_compile(
                lambda: jax.jit(rt["sm"], keep_unused=True)
                .lower(*args).compile())
        except Exception:
            rt["sharded"] = jax.jit(rt["sm"], keep_unused=True)
    return _exec(rt)


def _exec(rt):
    outs = rt["sharded"](*rt["dev_in"], *rt["zo_dev"])
    e = np.asarray(outs[0])            # [NCORES*K, TOK]; blocks until done
    return [e[c * K:(c + 1) * K] for c in range(NCORES)]


def kernel_rerun(n=1):
    """Execute the compiled kernel n times back-to-back (one sync at the
    end) and return the last run's per-core outputs."""
    rt = _CACHE["rt"]
    outs = None
    for _ in range(n):
        outs = rt["sharded"](*rt["dev_in"], *rt["zo_dev"])
    e = np.asarray(outs[0])
    return [e[c * K:(c + 1) * K] for c in range(NCORES)]


def kernel(emb, w_ih_f, w_hh_f, b_ih_f, b_hh_f, w_ih_b, w_hh_b, b_ih_b,
           b_hh_b, fc1_w, fc1_b, fc2_w, fc2_b, start_trans, trans, end_trans,
           tokens, tags, mask):
    shared = _prep_shared(emb, w_ih_f, w_hh_f, b_ih_f, b_hh_f, w_ih_b,
                          w_hh_b, b_ih_b, b_hh_b, fc1_w, fc1_b, fc2_w, fc2_b)
    tokens = np.asarray(tokens)
    in_maps = []
    for c in range(NCORES):
        tk = tokens[c * BC:(c + 1) * BC, :].astype(np.int32)  # [BC, T]
        tk = tk.T.reshape(T * BC, 1).copy()                   # t-major
        in_maps.append({**shared, "toks": tk})

    core_emis = _run_device(in_maps)

    emis = np.zeros((T, B, K), np.float32)
    for c in range(NCORES):
        e = np.asarray(core_emis[c])                          # [K, T*BC]
        emis[:, c * BC:(c + 1) * BC, :] = (
            e.reshape(K, T, BC).transpose(1, 2, 0))
    loss = _crf_host(emis, tags, mask, start_trans, trans, end_trans)
    return np.float32(loss)


# revision 12
# speedup vs baseline: 1.8518x; 1.8518x over previous
"""BiLSTM-CRF forward loss on 8 Trainium2 NeuronCores.

Data-parallel: batch 64 -> 8 sequences per core. Each core runs
embedding gather -> BiLSTM(T=512,H=256) -> fc1(32)+relu -> fc2(4),
and outputs its emissions [4, T*8]. The tiny CRF dynamic program
(O(T*B*K^2), K=4) and the final mean run on host in numpy.

v2: FP8 DoubleRow matmuls (256-deep contraction per instruction) for
the recurrent, input and fc1 weights; the xg add is folded into PSUM
accumulation via an fp8 identity matmul; tanh(g) is computed as
2*sigmoid(2g)-1 with the 2x folded into the weights and the affine
fixup fused into the DVE c-update, so one sigmoid activation covers
all 64 gate columns; the h-gate multiply runs on the Pool engine.
"""

import sys
for _p in ("/opt/trn_rl_repo", "/root/.axon_site/_ro/trn_rl_repo"):
    if _p not in sys.path:
        sys.path.insert(0, _p)
import numpy as np
from ml_dtypes import bfloat16

import concourse.bass as bass
import concourse.bacc as bacc
import concourse.mybir as mybir
from concourse.tile import TileContext
from concourse import bass_utils

B, T, E, H, V, K = 64, 512, 300, 256, 50000, 4
NCORES = 8
BC = B // NCORES          # 8 sequences per core
EP = 304                  # E padded to 304; row 300 = ones (bias trick)
G4H = 4 * H               # 1024
F32 = mybir.dt.float32
BF16 = mybir.dt.bfloat16
I32 = mybir.dt.int32
FP8 = mybir.dt.float8e4
AF = mybir.ActivationFunctionType
ALU = mybir.AluOpType
DR = mybir.MatmulPerfMode.DoubleRow


BUILD_KWARGS = {}


def build_bass(t_steps=T, bc=BC, parts="012f", dr=True):
    TOK = t_steps * bc
    nc = bacc.Bacc()

    # ---- DRAM parameters ----
    emb_aug = nc.dram_tensor("emb_aug", [V, EP], BF16, kind="ExternalInput")
    toks = nc.dram_tensor("toks", [TOK, 1], I32, kind="ExternalInput")
    # DoubleRow-packed input weights: rows 0:256 -> [128, 2*G4H], tail 48
    wxf = nc.dram_tensor("wxf", [128, 2 * G4H], FP8, kind="ExternalInput")
    wxb = nc.dram_tensor("wxb", [128, 2 * G4H], FP8, kind="ExternalInput")
    wxf_t = nc.dram_tensor("wxf_t", [48, G4H], FP8, kind="ExternalInput")
    wxb_t = nc.dram_tensor("wxb_t", [48, G4H], FP8, kind="ExternalInput")
    whf = nc.dram_tensor("whf", [128, 2 * G4H], FP8, kind="ExternalInput")
    whb = nc.dram_tensor("whb", [128, 2 * G4H], FP8, kind="ExternalInput")
    fc1w = nc.dram_tensor("fc1w", [128, 4 * 32], FP8, kind="ExternalInput")
    fc1b = nc.dram_tensor("fc1b", [32, 1], F32, kind="ExternalInput")
    fc2w = nc.dram_tensor("fc2w", [32, K], BF16, kind="ExternalInput")
    fc2b = nc.dram_tensor("fc2b", [K, 1], F32, kind="ExternalInput")
    iden = nc.dram_tensor("iden", [128, 128], BF16, kind="ExternalInput")
    iden8 = nc.dram_tensor("iden8", [128, 128], FP8, kind="ExternalInput")
    out = nc.dram_tensor("out", [K, TOK], F32, kind="ExternalOutput")

    n_ttile = TOK // 128          # token tiles of 128
    n_n512 = TOK // 512           # 512-wide token chunks
    ek = [(0, 128), (128, 128), (256, 48)]   # E-chunks for transpose

    with TileContext(nc) as tc:
        with tc.tile_pool(name="const", bufs=1) as constp, \
             tc.tile_pool(name="persist", bufs=1) as pp:
            # constants in SBUF
            id_sb = constp.tile([128, 128], BF16, tag="iden")
            nc.sync.dma_start(id_sb[:], iden[:])
            id8_sb = constp.tile([128, 128], FP8, tag="iden8")
            nc.sync.dma_start(id8_sb[:], iden8[:])
            wx_sb, wxt_sb, wh_sb = {}, {}, {}
            for d, src, srct, srch in (("f", wxf, wxf_t, whf),
                                       ("b", wxb, wxb_t, whb)):
                w = constp.tile([128, 2 * G4H], FP8, tag=f"wx{d}")
                nc.sync.dma_start(w[:], src[:])
                wx_sb[d] = w
                w = constp.tile([48, G4H], FP8, tag=f"wxt{d}")
                nc.sync.dma_start(w[:], srct[:])
                wxt_sb[d] = w
                w = constp.tile([128, 2 * G4H], FP8, tag=f"wh{d}")
                nc.sync.dma_start(w[:], srch[:])
                wh_sb[d] = w
            fc1w_sb = constp.tile([128, 4 * 32], FP8, tag="fc1w")
            nc.sync.dma_start(fc1w_sb[:], fc1w[:])
            fc2w_sb = constp.tile([32, K], BF16, tag="fc2w")
            nc.sync.dma_start(fc2w_sb[:], fc2w[:])
            fc1b_sb = constp.tile([32, 1], F32, tag="fc1b")
            nc.sync.dma_start(fc1b_sb[:], fc1b[:])
            fc2b_sb = constp.tile([K, 1], F32, tag="fc2b")
            nc.sync.dma_start(fc2b_sb[:], fc2b[:])

            # persistent activations
            # xg layout: [128, 8 gate-chunks * TOK], col = mu*TOK + t*bc + b
            # gate chunk order (host permutes): i,i,f,f,o,o,g,g
            xg = {d: pp.tile([128, 8 * TOK], FP8, tag=f"xg{d}", name=f"xg{d}")
                  for d in "fb"}
            # h layout: [128, 2 hid-chunks * TOK] fp8, col = k*TOK + t*bc + b
            hT = {d: pp.tile([128, 2 * TOK], FP8, tag=f"h{d}", name=f"h{d}")
                  for d in "fb"}

            with tc.tile_pool(name="xt", bufs=1) as xtp, \
                 tc.tile_pool(name="xrp", bufs=2) as xrp, \
                 tc.tile_pool(name="rec", bufs=1) as recp, \
                 tc.tile_pool(name="st", bufs=3) as stp, \
                 tc.tile_pool(name="fc", bufs=1) as fcp, \
                 tc.tile_pool(name="emo", bufs=2) as emop, \
                 tc.tile_pool(name="ps0", bufs=2, space="PSUM") as ps0p, \
                 tc.tile_pool(name="pst", bufs=2, space="PSUM") as pstp, \
                 tc.tile_pool(name="ps2", bufs=2, space="PSUM") as ps2p:
                # ---------- phase 0: gather + transpose -> xT (fp8) --------
                # xT01: k-tile pair (rows 0:128 | 128:256) for DoubleRow rhs
                xT01 = xtp.tile([128, 2 * TOK], FP8, tag="xT01", name="xT01")
                xT2 = xtp.tile([48, TOK], FP8, tag="xT2", name="xT2")
                idx_all = xtp.tile([128, n_ttile], I32, tag="idx_all")
                nc.gpsimd.dma_start(
                    idx_all[:],
                    toks[:].rearrange("(i p) one -> p (i one)", p=128))
                GC = min(8, n_ttile)        # token tiles per gather chunk
                if "0" not in parts:
                    GC = 0
                for c0 in range(0, n_ttile, GC if GC else n_ttile + 1):
                    xr = xrp.tile([128, GC * EP], BF16, tag="xr", name="xr")
                    for j in range(GC):
                        i = c0 + j
                        nc.gpsimd.indirect_dma_start(
                            out=xr[:, j * EP:(j + 1) * EP], out_offset=None,
                            in_=emb_aug[:],
                            in_offset=bass.IndirectOffsetOnAxis(
                                ap=idx_all[:, i:i + 1], axis=0),
                        )
                    for j in range(GC):
                        i = c0 + j
                        for ki, (r0, rn) in enumerate(ek):
                            pt = pstp.tile([128, 128], BF16, tag="tp")
                            nc.tensor.transpose(
                                out=pt[:rn, :],
                                in_=xr[:, j * EP + r0:j * EP + r0 + rn],
                                identity=id_sb[:])
                            if ki < 2:
                                dst = xT01[:, ki * TOK + i * 128:
                                           ki * TOK + (i + 1) * 128]
                            else:
                                dst = xT2[:, i * 128:(i + 1) * 128]
                            nc.vector.tensor_copy(out=dst, in_=pt[:rn, :])

                xT01v = xT01[:].rearrange("p (k t) -> p k t", k=2)
                for d in "fb":
                    # ---- phase 1: xg = wx^T @ x (DoubleRow + 48-row tail) --
                    wxv = wx_sb[d][:].rearrange("p (k g) -> p k g", k=2)
                    for mu in range(8):
                        for n in range(n_n512):
                            ps = ps0p.tile([128, 512], F32, tag="mm")
                            if dr:
                                nc.tensor.matmul(
                                    ps[:],
                                    lhsT=wxv[:, :, mu * 128:(mu + 1) * 128],
                                    rhs=xT01v[:, :, n * 512:(n + 1) * 512],
                                    start=True, stop=False,
                                    perf_mode=DR, skip_group_check=True)
                            else:
                                for ki in range(2):
                                    nc.tensor.matmul(
                                        ps[:],
                                        lhsT=wxv[:, ki, mu * 128:(mu + 1) * 128],
                                        rhs=xT01v[:, ki, n * 512:(n + 1) * 512],
                                        start=(ki == 0), stop=False,
                                        skip_group_check=True)
                            nc.tensor.matmul(
                                ps[:],
                                lhsT=wxt_sb[d][:, mu * 128:(mu + 1) * 128],
                                rhs=xT2[:, n * 512:(n + 1) * 512],
                                start=False, stop=True,
                                skip_group_check=True)
                            dst = xg[d][:, mu * TOK + n * 512:
                                        mu * TOK + (n + 1) * 512]
                            if n % 2 == 0:
                                nc.scalar.copy(out=dst, in_=ps[:])
                            else:
                                nc.vector.tensor_copy(out=dst, in_=ps[:])

                # -------- phase 2: recurrence, fwd+bwd interleaved ----------
                # sa cols per dir: [i(16) f(16) o(16) g2(16)] (ki-major pairs)
                c_st = {d: recp.tile([128, 2 * bc], F32, tag=f"c{d}",
                                     name=f"c{d}") for d in "fb"}
                hTv = {d: hT[d][:].rearrange("p (k t) -> p k t", k=2)
                       for d in "fb"}
                whv = {d: wh_sb[d][:].rearrange("p (k g) -> p k g", k=2)
                       for d in "fb"}
                for step in range(t_steps):
                    for d in "fb":
                        t = step if d == "f" else t_steps - 1 - step
                        first = step == 0
                        ps = ps2p.tile([128, 8 * bc], F32, tag=f"ps{d}",
                                       name=f"ps{d}")
                        xga = xg[d][:].rearrange(
                            "p (m t) -> p m t", m=8)[:, :, t * bc:(t + 1) * bc]
                        # xg add via fp8 identity matmul (starts accumulation)
                        nc.tensor.matmul(
                            ps[:], lhsT=id8_sb[:], rhs=xga,
                            start=True, stop=first, skip_group_check=True)
                        if not first:
                            tprev = t - 1 if d == "f" else t + 1
                            for mu in range(8):
                                if dr:
                                    nc.tensor.matmul(
                                        ps[:, mu * bc:(mu + 1) * bc],
                                        lhsT=whv[d][:, :, mu * 128:(mu + 1) * 128],
                                        rhs=hTv[d][:, :, tprev * bc:(tprev + 1) * bc],
                                        start=False, stop=True,
                                        perf_mode=DR, skip_group_check=True)
                                else:
                                    for ki in range(2):
                                        nc.tensor.matmul(
                                            ps[:, mu * bc:(mu + 1) * bc],
                                            lhsT=whv[d][:, ki, mu * 128:(mu + 1) * 128],
                                            rhs=hTv[d][:, ki, tprev * bc:(tprev + 1) * bc],
                                            start=False, stop=(ki == 1),
                                            skip_group_check=True)
                        sa = stp.tile([128, 8 * bc], F32, tag=f"sa{d}",
                                      name=f"sa{d}")
                        nc.scalar.activation(sa[:], ps[:], AF.Sigmoid)
                        # c = sig_f*c + sig_i*(2*sig_g2 - 1)
                        t2 = stp.tile([128, 2 * bc], F32, tag=f"t2{d}",
                                      name=f"t2{d}")
                        nc.vector.scalar_tensor_tensor(
                            out=t2[:], in0=sa[:, 6 * bc:8 * bc], scalar=-0.5,
                            in1=sa[:, 0:2 * bc], op0=ALU.add, op1=ALU.mult)
                        if first:
                            nc.vector.tensor_scalar_mul(
                                c_st[d][:], t2[:], 2.0)
                        else:
                            t1 = stp.tile([128, 2 * bc], F32, tag=f"t1{d}",
                                          name=f"t1{d}")
                            nc.vector.tensor_tensor(
                                out=t1[:], in0=sa[:, 2 * bc:4 * bc],
                                in1=c_st[d][:], op=ALU.mult)
                            nc.vector.scalar_tensor_tensor(
                                out=c_st[d][:], in0=t2[:], scalar=2.0,
                                in1=t1[:], op0=ALU.mult, op1=ALU.add)
                        tcl = stp.tile([128, 2 * bc], F32, tag=f"tc{d}",
                                       name=f"tc{d}")
                        nc.scalar.activation(tcl[:], c_st[d][:], AF.Tanh)
                        # h = sig_o * tanh(c), fp8 out
                        nc.vector.tensor_tensor(
                            out=hTv[d][:, :, t * bc:(t + 1) * bc],
                            in0=sa[:, 4 * bc:6 * bc].rearrange(
                                "p (k c) -> p k c", k=2),
                            in1=tcl[:].rearrange("p (k c) -> p k c", k=2),
                            op=ALU.mult)

                # ---------- phase 3: fc1 + relu, fc2 + bias, out ----------
                z = fcp.tile([32, TOK], BF16, tag="z")
                fc1v = fc1w_sb[:].rearrange("p (dk c) -> p dk c", dk=4)
                n_n512_f = n_n512 if "f" in parts else 0
                for n in range(n_n512_f):
                    ps = ps0p.tile([32, 512], F32, tag="mm")
                    for di, d in enumerate("fb"):
                        if dr:
                            nc.tensor.matmul(
                                ps[:],
                                lhsT=fc1v[:, 2 * di:2 * di + 2, :],
                                rhs=hTv[d][:, :, n * 512:(n + 1) * 512],
                                start=(di == 0), stop=(di == 1),
                                perf_mode=DR, skip_group_check=True)
                        else:
                            for ki in range(2):
                                nc.tensor.matmul(
                                    ps[:],
                                    lhsT=fc1v[:, 2 * di + ki, :],
                                    rhs=hTv[d][:, ki, n * 512:(n + 1) * 512],
                                    start=(di == 0 and ki == 0),
                                    stop=(di == 1 and ki == 1),
                                    skip_group_check=True)
                    nc.scalar.activation(z[:, n * 512:(n + 1) * 512], ps[:],
                                         AF.Relu, bias=fc1b_sb[:, :1])
                for n in range(n_n512_f):
                    ps = ps0p.tile([K, 512], F32, tag="mm")
                    nc.tensor.matmul(ps[:], lhsT=fc2w_sb[:],
                                     rhs=z[:, n * 512:(n + 1) * 512],
                                     start=True, stop=True)
                    em = emop.tile([K, 512], F32, tag="em", name="em")
                    nc.vector.tensor_scalar_add(em[:], ps[:], fc2b_sb[:, :1])
                    nc.sync.dma_start(out[:, n * 512:(n + 1) * 512], em[:])
    nc.compile()
    return nc


def _pack_dr(m):
    """[256, N] -> DoubleRow-packed [128, 2*N]: out[p, k*N+j] = m[k*128+p, j]"""
    n = m.shape[1]
    return m.reshape(2, 128, n).transpose(1, 0, 2).reshape(128, 2 * n)


def _prep_shared(emb, w_ih_f, w_hh_f, b_ih_f, b_hh_f, w_ih_b, w_hh_b,
                 b_ih_b, b_hh_b, fc1_w, fc1_b, fc2_w, fc2_b):
    f32 = np.float32
    fp8 = mybir.dt.np(FP8)
    emb_aug = np.zeros((V, EP), f32)
    emb_aug[:, :E] = np.asarray(emb, f32)
    emb_aug[0, :E] = 0.0
    emb_aug[:, E] = 1.0

    perm = np.r_[0:512, 768:1024, 512:768]  # i,f,g,o -> i,f,o,g

    def wx(w_ih, b_ih, b_hh):
        m = np.zeros((EP, G4H), f32)
        m[:E, :] = np.asarray(w_ih, f32).T
        m[E, :] = np.asarray(b_ih, f32) + np.asarray(b_hh, f32)
        m = m[:, perm]
        m[:, 768:] *= 2.0          # tanh(g) = 2*sigmoid(2g) - 1
        return m

    def wh(w_hh):
        m = np.asarray(w_hh, f32).T[:, perm].copy()
        m[:, 768:] *= 2.0
        return m

    wxf_full = wx(w_ih_f, b_ih_f, b_hh_f)
    wxb_full = wx(w_ih_b, b_ih_b, b_hh_b)
    fc1 = np.asarray(fc1_w, f32).T        # [512, 32]
    fc1_pk = np.concatenate(
        [_pack_dr(fc1[0:256]), _pack_dr(fc1[256:512])], axis=1)  # [128, 4*32]

    return dict(
        emb_aug=emb_aug.astype(bfloat16).copy(),
        wxf=_pack_dr(wxf_full[0:256]).astype(fp8).copy(),
        wxb=_pack_dr(wxb_full[0:256]).astype(fp8).copy(),
        wxf_t=wxf_full[256:304].astype(fp8).copy(),
        wxb_t=wxb_full[256:304].astype(fp8).copy(),
        whf=_pack_dr(wh(w_hh_f)).astype(fp8).copy(),
        whb=_pack_dr(wh(w_hh_b)).astype(fp8).copy(),
        fc1w=fc1_pk.astype(fp8).copy(),
        fc1b=np.asarray(fc1_b, f32).reshape(32, 1).copy(),
        fc2w=np.asarray(fc2_w, f32).T.astype(bfloat16).copy(),
        fc2b=np.asarray(fc2_b, f32).reshape(K, 1).copy(),
        iden=np.eye(128, dtype=f32).astype(bfloat16).copy(),
        iden8=np.eye(128, dtype=f32).astype(fp8).copy(),
    )


def _crf_host(emis, tags, mask, start_trans, trans, end_trans):
    # emis: [T, B, K] f32; exact forward algorithm in float64 on host
    emis = emis.astype(np.float64)
    trans = np.asarray(trans, np.float64)
    start = np.asarray(start_trans, np.float64)
    end = np.asarray(end_trans, np.float64)
    tags = np.asarray(tags, np.int64)
    m = np.asarray(mask, np.float64).T           # [T, B]
    tg = tags.T                                  # [T, B]
    Bsz = emis.shape[1]
    bidx = np.arange(Bsz)

    score = start[tg[0]] + emis[0, bidx, tg[0]]
    for t in range(1, emis.shape[0]):
        score = score + (trans[tg[t - 1], tg[t]] + emis[t, bidx, tg[t]]) * m[t]
    seq_ends = np.asarray(mask, np.int64).sum(1) - 1
    score = score + end[tg[seq_ends, bidx]]

    alpha = start[None, :] + emis[0]
    for t in range(1, emis.shape[0]):
        nxt = alpha[:, :, None] + trans[None] + emis[t][:, None, :]
        mx = nxt.max(axis=1)
        nxt = mx + np.log(np.exp(nxt - mx[:, None, :]).sum(axis=1))
        alpha = np.where(m[t][:, None] > 0, nxt, alpha)
    av = alpha + end[None, :]
    mx = av.max(axis=1)
    logZ = mx + np.log(np.exp(av - mx[:, None]).sum(axis=1))
    return -(score - logZ).mean()


_CACHE = {}


def _make_runner():
    import jax
    from jax.sharding import Mesh, PartitionSpec, NamedSharding
    try:
        from jax.experimental.shard_map import shard_map
    except ImportError:
        from jax import shard_map
    from concourse import bass2jax
    from concourse.bass2jax import _bass_exec_p, partition_id_tensor

    nc = build_bass(**BUILD_KWARGS)
    bass2jax.install_neuronx_cc_hook()
    partition_name = (nc.partition_id_tensor.name
                      if nc.partition_id_tensor else None)
    in_names, out_names, out_avals, zero_outs = [], [], [], []
    for alloc in nc.m.functions[0].allocations:
        if not isinstance(alloc, mybir.MemoryLocationSet):
            continue
        name = alloc.memorylocations[0].name
        if alloc.kind == "ExternalInput":
            if name != partition_name:
                in_names.append(name)
        elif alloc.kind == "ExternalOutput":
            shape = tuple(alloc.tensor_shape)
            dtype = mybir.dt.np(alloc.dtype)
            out_names.append(name)
            out_avals.append(jax.core.ShapedArray(shape, dtype))
            zero_outs.append(np.zeros(shape, dtype))
    n_params = len(in_names)
    in_names_all = in_names + out_names
    if partition_name is not None:
        in_names_all.append(partition_name)

    def _body(*args):
        operands = list(args)
        if partition_name is not None:
            operands.append(partition_id_tensor())
        return tuple(_bass_exec_p.bind(
            *operands, out_avals=tuple(out_avals),
            in_names=tuple(in_names_all), out_names=tuple(out_names),
            lowering_input_output_aliases=(),
            sim_require_finite=True, sim_require_nnan=True, nc=nc))

    devices = jax.devices()[:NCORES]
    mesh = Mesh(np.asarray(devices), ("core",))
    sh = NamedSharding(mesh, PartitionSpec("core"))
    # The kernel writes every element of its outputs, so the zero output
    # buffers are NOT donated: they are uploaded once and reused by every
    # execution (saves one ~70ms host->device sync per run).
    sm = shard_map(_body, mesh=mesh,
                   in_specs=(PartitionSpec("core"),) * (n_params + len(out_names)),
                   out_specs=(PartitionSpec("core"),) * len(out_names),
                   check_rep=False)
    return dict(jax=jax, sm=sm, sh=sh, in_names=in_names,
                out_names=out_names, zero_outs=zero_outs)


def _run_device(in_maps):
    if "rt" not in _CACHE:
        _CACHE["rt"] = _make_runner()
    rt = _CACHE["rt"]
    jax = rt["jax"]
    from concourse.bass2jax import fast_dispatch_compile
    concat_in = [np.concatenate([np.asarray(m[n]) for m in in_maps], 0)
                 for n in rt["in_names"]]
    rt["dev_in"] = [jax.device_put(a, rt["sh"]) for a in concat_in]
    rt["zo_dev"] = [jax.device_put(np.concatenate([z] * NCORES, 0), rt["sh"])
                    for z in rt["zero_outs"]]
    if "sharded" not in rt:
        args = tuple(rt["dev_in"]) + tuple(rt["zo_dev"])
        try:
            rt["sharded"] = fast_dispatch_compile(
                lambda: jax.jit(rt["sm"], keep_unused=True)
                .lower(*args).compile())
        except Exception:
            rt["sharded"] = jax.jit(rt["sm"], keep_unused=True)
    return _exec(rt)


def _exec(rt):
    outs = rt["sharded"](*rt["dev_in"], *rt["zo_dev"])
    e = np.asarray(outs[0])            # [NCORES*K, TOK]; blocks until done
    return [e[c * K:(c + 1) * K] for c in range(NCORES)]


def kernel_rerun(n=1):
    """Execute the compiled kernel n times back-to-back (one sync at the
    end) and return the last run's per-core outputs."""
    rt = _CACHE["rt"]
    outs = None
    for _ in range(n):
        outs = rt["sharded"](*rt["dev_in"], *rt["zo_dev"])
    e = np.asarray(outs[0])
    return [e[c * K:(c + 1) * K] for c in range(NCORES)]


def kernel(emb, w_ih_f, w_hh_f, b_ih_f, b_hh_f, w_ih_b, w_hh_b, b_ih_b,
           b_hh_b, fc1_w, fc1_b, fc2_w, fc2_b, start_trans, trans, end_trans,
           tokens, tags, mask):
    shared = _prep_shared(emb, w_ih_f, w_hh_f, b_ih_f, b_hh_f, w_ih_b,
                          w_hh_b, b_ih_b, b_hh_b, fc1_w, fc1_b, fc2_w, fc2_b)
    tokens = np.asarray(tokens)
    in_maps = []
    for c in range(NCORES):
        tk = tokens[c * BC:(c + 1) * BC, :].astype(np.int32)  # [BC, T]
        tk = tk.T.reshape(T * BC, 1).copy()                   # t-major
        in_maps.append({**shared, "toks": tk})

    core_emis = _run_device(in_maps)

    emis = np.zeros((T, B, K), np.float32)
    for c in range(NCORES):
        e = np.asarray(core_emis[c])                          # [K, T*BC]
        emis[:, c * BC:(c + 1) * BC, :] = (
            e.reshape(K, T, BC).transpose(1, 2, 0))
    loss = _crf_host(emis, tags, mask, start_trans, trans, end_trans)
    return np.float32(loss)


# revision 24
# speedup vs baseline: 2.2795x; 1.2310x over previous
"""BiLSTM-CRF forward loss on 8 Trainium2 NeuronCores.

Data-parallel: batch 64 -> 8 sequences per core. Each core runs
embedding gather -> BiLSTM(T=512,H=256) -> fc1(32)+relu -> fc2(4),
and outputs its emissions [4, T*8]. The tiny CRF dynamic program
(O(T*B*K^2), K=4) and the final mean run on host in numpy.

v2: FP8 DoubleRow matmuls (256-deep contraction per instruction) for
the recurrent, input and fc1 weights; the xg add is folded into PSUM
accumulation via an fp8 identity matmul; tanh(g) is computed as
2*sigmoid(2g)-1 with the 2x folded into the weights and the affine
fixup fused into the DVE c-update, so one sigmoid activation covers
all 64 gate columns; the h-gate multiply runs on the Pool engine.
"""

import sys
for _p in ("/opt/trn_rl_repo", "/root/.axon_site/_ro/trn_rl_repo"):
    if _p not in sys.path:
        sys.path.insert(0, _p)
import numpy as np
from ml_dtypes import bfloat16

import concourse.bass as bass
import concourse.bacc as bacc
import concourse.mybir as mybir
from concourse.tile import TileContext
from concourse import bass_utils

B, T, E, H, V, K = 64, 512, 300, 256, 50000, 4
NCORES = 8
BC = B // NCORES          # 8 sequences per core
EP = 304                  # E padded to 304; row 300 = ones (bias trick)
G4H = 4 * H               # 1024
F32 = mybir.dt.float32
BF16 = mybir.dt.bfloat16
I32 = mybir.dt.int32
FP8 = mybir.dt.float8e4
AF = mybir.ActivationFunctionType
ALU = mybir.AluOpType
DR = mybir.MatmulPerfMode.DoubleRow


BUILD_KWARGS = {}


def build_bass(t_steps=T, bc=BC, parts="012f", dr=True, dr2=False,
               rec_steps=None, elem=True):
    TOK = t_steps * bc
    nc = bacc.Bacc()

    # ---- DRAM parameters ----
    emb_aug = nc.dram_tensor("emb_aug", [V, EP], BF16, kind="ExternalInput")
    toks = nc.dram_tensor("toks", [TOK, 1], I32, kind="ExternalInput")
    # DoubleRow-packed input weights: rows 0:256 -> [128, 2*G4H], tail 48
    wxf = nc.dram_tensor("wxf", [128, 2 * G4H], FP8, kind="ExternalInput")
    wxb = nc.dram_tensor("wxb", [128, 2 * G4H], FP8, kind="ExternalInput")
    wxf_t = nc.dram_tensor("wxf_t", [48, G4H], FP8, kind="ExternalInput")
    wxb_t = nc.dram_tensor("wxb_t", [48, G4H], FP8, kind="ExternalInput")
    whf = nc.dram_tensor("whf", [128, 2 * G4H], FP8, kind="ExternalInput")
    whb = nc.dram_tensor("whb", [128, 2 * G4H], FP8, kind="ExternalInput")
    fc1w = nc.dram_tensor("fc1w", [128, 4 * 32], FP8, kind="ExternalInput")
    fc1b = nc.dram_tensor("fc1b", [32, 1], F32, kind="ExternalInput")
    fc2w = nc.dram_tensor("fc2w", [32, K], BF16, kind="ExternalInput")
    fc2b = nc.dram_tensor("fc2b", [K, 1], F32, kind="ExternalInput")
    iden = nc.dram_tensor("iden", [128, 128], BF16, kind="ExternalInput")
    iden8 = nc.dram_tensor("iden8", [128, 128], FP8, kind="ExternalInput")
    out = nc.dram_tensor("out", [K, TOK], F32, kind="ExternalOutput")

    n_ttile = TOK // 128          # token tiles of 128
    n_n512 = TOK // 512           # 512-wide token chunks
    ek = [(0, 128), (128, 128), (256, 48)]   # E-chunks for transpose

    with TileContext(nc) as tc:
        with tc.tile_pool(name="const", bufs=1) as constp, \
             tc.tile_pool(name="persist", bufs=1) as pp:
            # constants in SBUF
            id_sb = constp.tile([128, 128], BF16, tag="iden")
            nc.sync.dma_start(id_sb[:], iden[:])
            id8_sb = constp.tile([128, 128], FP8, tag="iden8")
            nc.sync.dma_start(id8_sb[:], iden8[:])
            wx_sb, wxt_sb, wh_sb = {}, {}, {}
            for d, src, srct, srch in (("f", wxf, wxf_t, whf),
                                       ("b", wxb, wxb_t, whb)):
                w = constp.tile([128, 2 * G4H], FP8, tag=f"wx{d}")
                nc.sync.dma_start(w[:], src[:])
                wx_sb[d] = w
                w = constp.tile([48, G4H], FP8, tag=f"wxt{d}")
                nc.sync.dma_start(w[:], srct[:])
                wxt_sb[d] = w
                w = constp.tile([128, 2 * G4H], FP8, tag=f"wh{d}")
                nc.sync.dma_start(w[:], srch[:])
                wh_sb[d] = w
            fc1w_sb = constp.tile([128, 4 * 32], FP8, tag="fc1w")
            nc.sync.dma_start(fc1w_sb[:], fc1w[:])
            fc2w_sb = constp.tile([32, K], BF16, tag="fc2w")
            nc.sync.dma_start(fc2w_sb[:], fc2w[:])
            fc1b_sb = constp.tile([32, 1], F32, tag="fc1b")
            nc.sync.dma_start(fc1b_sb[:], fc1b[:])
            fc2b_sb = constp.tile([K, 1], F32, tag="fc2b")
            nc.sync.dma_start(fc2b_sb[:], fc2b[:])

            # persistent activations
            # xg lives in per-(dir,window) tiles xgn below; gate chunk order
            # (host permutes): i,i,f,f,o,o,g,g
            # h layout: [128, 2 hid-chunks * TOK] fp8, col = k*TOK + t*bc + b
            hT = {d: pp.tile([128, 2 * TOK], FP8, tag=f"h{d}", name=f"h{d}")
                  for d in "fb"}

            with tc.tile_pool(name="xt", bufs=1) as xtp, \
                 tc.tile_pool(name="xrp", bufs=8) as xrp, \
                 tc.tile_pool(name="rec", bufs=1) as recp, \
                 tc.tile_pool(name="st", bufs=3) as stp, \
                 tc.tile_pool(name="fc", bufs=1) as fcp, \
                 tc.tile_pool(name="emo", bufs=2) as emop, \
                 tc.tile_pool(name="ps0", bufs=2, space="PSUM") as ps0p, \
                 tc.tile_pool(name="pst", bufs=2, space="PSUM") as pstp, \
                 tc.tile_pool(name="ps2", bufs=2, space="PSUM") as ps2p:
                # ---------- phase 0+1, interleaved into phase 2 ------------
                # Token windows of WSTEP steps; xg for window w of dir d is
                # produced (gather -> transpose -> matmul) as PE filler work
                # during window w-1 of the recurrence, keeping the PE busy
                # (and its clock ramped) while the LSTM chain runs.
                WN = n_n512                     # windows (8)
                WSTEP = t_steps // WN           # steps per window (64)
                WTOK = WSTEP * bc               # tokens per window (512)
                TT = WTOK // 128                # token tiles per window (4)
                # xT01: k-tile pair (rows 0:128 | 128:256) for DoubleRow rhs
                xT01 = xtp.tile([128, 2 * TOK], FP8, tag="xT01", name="xT01")
                xT2 = xtp.tile([48, TOK], FP8, tag="xT2", name="xT2")
                idx_all = xtp.tile([128, n_ttile], I32, tag="idx_all")
                nc.gpsimd.dma_start(
                    idx_all[:],
                    toks[:].rearrange("(i p) one -> p (i one)", p=128))
                xT01v = xT01[:].rearrange("p (k t) -> p k t", k=2)
                wxv = {d: wx_sb[d][:].rearrange("p (k g) -> p k g", k=2)
                       for d in "fb"}
                xr_of = {}

                def emit_gather(i):
                    xr = xrp.tile([128, EP], BF16, tag="xr", name="xr")
                    xr_of[i] = xr
                    nc.gpsimd.indirect_dma_start(
                        out=xr[:], out_offset=None,
                        in_=emb_aug[:],
                        in_offset=bass.IndirectOffsetOnAxis(
                            ap=idx_all[:, i:i + 1], axis=0),
                    )

                def emit_transpose(i):
                    xr = xr_of.pop(i)
                    for ki, (r0, rn) in enumerate(ek):
                        pt = pstp.tile([128, 128], BF16, tag="tp")
                        nc.tensor.transpose(
                            out=pt[:rn, :], in_=xr[:, r0:r0 + rn],
                            identity=id_sb[:])
                        if ki < 2:
                            dst = xT01[:, ki * TOK + i * 128:
                                       ki * TOK + (i + 1) * 128]
                        else:
                            dst = xT2[:, i * 128:(i + 1) * 128]
                        nc.vector.tensor_copy(out=dst, in_=pt[:rn, :])

                xgn = {(d, n): pp.tile([128, 8 * WTOK], FP8, tag=f"xg{d}{n}",
                                       name=f"xg{d}{n}")
                       for d in "fb" for n in range(WN)}

                def emit_xg(d, n, mu):
                    ps = ps0p.tile([128, 512], F32, tag="mm")
                    if dr:
                        nc.tensor.matmul(
                            ps[:],
                            lhsT=wxv[d][:, :, mu * 128:(mu + 1) * 128],
                            rhs=xT01v[:, :, n * WTOK:(n + 1) * WTOK],
                            start=True, stop=False,
                            perf_mode=DR, skip_group_check=True)
                    else:
                        for ki in range(2):
                            nc.tensor.matmul(
                                ps[:],
                                lhsT=wxv[d][:, ki, mu * 128:(mu + 1) * 128],
                                rhs=xT01v[:, ki, n * WTOK:(n + 1) * WTOK],
                                start=(ki == 0), stop=False,
                                skip_group_check=True)
                    nc.tensor.matmul(
                        ps[:],
                        lhsT=wxt_sb[d][:, mu * 128:(mu + 1) * 128],
                        rhs=xT2[:, n * WTOK:(n + 1) * WTOK],
                        start=False, stop=True,
                        skip_group_check=True)
                    dst = xgn[(d, n)][:, mu * WTOK:(mu + 1) * WTOK]
                    if (mu + n) % 2 == 0:
                        nc.scalar.copy(out=dst, in_=ps[:])
                    else:
                        nc.vector.tensor_copy(out=dst, in_=ps[:])

                # --- prelude: first f window and last b window ---
                done_tr = set()
                pre_f = list(range(TT))
                pre_b = list(range(TT * (WN - 1), TT * WN))
                for i in pre_f + pre_b:
                    emit_gather(i)
                for i in pre_f + pre_b:
                    emit_transpose(i)
                    done_tr.add(i)
                for mu in range(8):
                    emit_xg("f", 0, mu)
                for mu in range(8):
                    emit_xg("b", WN - 1, mu)

                # --- filler schedule for windows 0..WN-2 ---
                go = []          # remaining gathers, need-order
                for w in range(1, WN):
                    for i in range(TT * w, TT * w + TT):
                        if i not in done_tr and i not in go:
                            go.append(i)
                    bb = WN - 1 - w
                    for i in range(TT * bb, TT * bb + TT):
                        if i not in done_tr and i not in go:
                            go.append(i)
                win_items = [[] for _ in range(WN)]
                win_items[0].extend(
                    (lambda i=i: emit_gather(i)) for i in go)
                for k in range(WN - 1):
                    fw, bw = k + 1, WN - 2 - k
                    for i in list(range(TT * fw, TT * fw + TT)) + \
                            list(range(TT * bw, TT * bw + TT)):
                        if i not in done_tr:
                            done_tr.add(i)
                            win_items[k].append(lambda i=i: emit_transpose(i))
                    for mu in range(8):
                        win_items[k].append(
                            lambda d="f", n=fw, mu=mu: emit_xg(d, n, mu))
                    for mu in range(8):
                        win_items[k].append(
                            lambda d="b", n=bw, mu=mu: emit_xg(d, n, mu))
                win_done = [0] * WN

                def drain_filler(step):
                    k = step // WSTEP
                    if k >= WN:
                        return
                    items = win_items[k]
                    target = ((step % WSTEP) + 1) * len(items)
                    target = -(-target // WSTEP)     # ceil
                    while win_done[k] < target:
                        items[win_done[k]]()
                        win_done[k] += 1

                # -------- phase 2: recurrence, fwd+bwd interleaved ----------
                # sa cols per dir: [i(16) f(16) o(16) g2(16)] (ki-major pairs)
                c_st = {d: recp.tile([128, 2 * bc], F32, tag=f"c{d}",
                                     name=f"c{d}") for d in "fb"}
                hTv = {d: hT[d][:].rearrange("p (k t) -> p k t", k=2)
                       for d in "fb"}
                whv = {d: wh_sb[d][:].rearrange("p (k g) -> p k g", k=2)
                       for d in "fb"}
                for step in range(t_steps if rec_steps is None else rec_steps):
                    for d in "fb":
                        t = step if d == "f" else t_steps - 1 - step
                        first = step == 0
                        ps = ps2p.tile([128, 8 * bc], F32, tag=f"ps{d}",
                                       name=f"ps{d}")
                        tw, wo = t // WSTEP, t % WSTEP
                        xga = xgn[(d, tw)][:].rearrange(
                            "p (m t) -> p m t", m=8)[:, :, wo * bc:(wo + 1) * bc]
                        # xg add via fp8 identity matmul (starts accumulation)
                        nc.tensor.matmul(
                            ps[:], lhsT=id8_sb[:], rhs=xga,
                            start=True, stop=first, skip_group_check=True)
                        if not first:
                            tprev = t - 1 if d == "f" else t + 1
                            for mu in range(8):
                                if dr2:
                                    nc.tensor.matmul(
                                        ps[:, mu * bc:(mu + 1) * bc],
                                        lhsT=whv[d][:, :, mu * 128:(mu + 1) * 128],
                                        rhs=hTv[d][:, :, tprev * bc:(tprev + 1) * bc],
                                        start=False, stop=True,
                                        perf_mode=DR, skip_group_check=True)
                                else:
                                    for ki in range(2):
                                        nc.tensor.matmul(
                                            ps[:, mu * bc:(mu + 1) * bc],
                                            lhsT=whv[d][:, ki, mu * 128:(mu + 1) * 128],
                                            rhs=hTv[d][:, ki, tprev * bc:(tprev + 1) * bc],
                                            start=False, stop=(ki == 1),
                                            skip_group_check=True)
                        if not elem:
                            # timing-probe mode: write hT straight from psum
                            nc.vector.tensor_copy(
                                out=hTv[d][:, :, t * bc:(t + 1) * bc],
                                in_=ps[:, 0:2 * bc].rearrange(
                                    "p (k c) -> p k c", k=2))
                            continue
                        sa = stp.tile([128, 8 * bc], F32, tag=f"sa{d}",
                                      name=f"sa{d}")
                        nc.scalar.activation(sa[:], ps[:], AF.Sigmoid)
                        # c = sig_f*c + sig_i*(2*sig_g2 - 1)
                        t2 = stp.tile([128, 2 * bc], F32, tag=f"t2{d}",
                                      name=f"t2{d}")
                        nc.vector.scalar_tensor_tensor(
                            out=t2[:], in0=sa[:, 6 * bc:8 * bc], scalar=-0.5,
                            in1=sa[:, 0:2 * bc], op0=ALU.add, op1=ALU.mult)
                        if first:
                            nc.vector.tensor_scalar_mul(
                                c_st[d][:], t2[:], 2.0)
                        else:
                            t1 = stp.tile([128, 2 * bc], F32, tag=f"t1{d}",
                                          name=f"t1{d}")
                            nc.vector.tensor_tensor(
                                out=t1[:], in0=sa[:, 2 * bc:4 * bc],
                                in1=c_st[d][:], op=ALU.mult)
                            nc.vector.scalar_tensor_tensor(
                                out=c_st[d][:], in0=t2[:], scalar=2.0,
                                in1=t1[:], op0=ALU.mult, op1=ALU.add)
                        tcl = stp.tile([128, 2 * bc], F32, tag=f"tc{d}",
                                       name=f"tc{d}")
                        nc.scalar.activation(tcl[:], c_st[d][:], AF.Tanh)
                        # h = sig_o * tanh(c), fp8 out
                        nc.vector.tensor_tensor(
                            out=hTv[d][:, :, t * bc:(t + 1) * bc],
                            in0=sa[:, 4 * bc:6 * bc].rearrange(
                                "p (k c) -> p k c", k=2),
                            in1=tcl[:].rearrange("p (k c) -> p k c", k=2),
                            op=ALU.mult)
                    drain_filler(step)

                # leftovers (short rec_steps ablations)
                for k in range(WN):
                    while win_done[k] < len(win_items[k]):
                        win_items[k][win_done[k]]()
                        win_done[k] += 1

                # ---------- phase 3: fc1 + relu, fc2 + bias, out ----------
                z = fcp.tile([32, TOK], BF16, tag="z")
                fc1v = fc1w_sb[:].rearrange("p (dk c) -> p dk c", dk=4)
                n_n512_f = n_n512 if "f" in parts else 0
                for n in range(n_n512_f):
                    ps = ps0p.tile([32, 512], F32, tag="mm")
                    for di, d in enumerate("fb"):
                        if dr:
                            nc.tensor.matmul(
                                ps[:],
                                lhsT=fc1v[:, 2 * di:2 * di + 2, :],
                                rhs=hTv[d][:, :, n * 512:(n + 1) * 512],
                                start=(di == 0), stop=(di == 1),
                                perf_mode=DR, skip_group_check=True)
                        else:
                            for ki in range(2):
                                nc.tensor.matmul(
                                    ps[:],
                                    lhsT=fc1v[:, 2 * di + ki, :],
                                    rhs=hTv[d][:, ki, n * 512:(n + 1) * 512],
                                    start=(di == 0 and ki == 0),
                                    stop=(di == 1 and ki == 1),
                                    skip_group_check=True)
                    nc.scalar.activation(z[:, n * 512:(n + 1) * 512], ps[:],
                                         AF.Relu, bias=fc1b_sb[:, :1])
                for n in range(n_n512_f):
                    ps = ps0p.tile([K, 512], F32, tag="mm")
                    nc.tensor.matmul(ps[:], lhsT=fc2w_sb[:],
                                     rhs=z[:, n * 512:(n + 1) * 512],
                                     start=True, stop=True)
                    em = emop.tile([K, 512], F32, tag="em", name="em")
                    nc.vector.tensor_scalar_add(em[:], ps[:], fc2b_sb[:, :1])
                    nc.sync.dma_start(out[:, n * 512:(n + 1) * 512], em[:])
    nc.compile()
    return nc


def _pack_dr(m):
    """[256, N] -> DoubleRow-packed [128, 2*N]: out[p, k*N+j] = m[k*128+p, j]"""
    n = m.shape[1]
    return m.reshape(2, 128, n).transpose(1, 0, 2).reshape(128, 2 * n)


def _prep_shared(emb, w_ih_f, w_hh_f, b_ih_f, b_hh_f, w_ih_b, w_hh_b,
                 b_ih_b, b_hh_b, fc1_w, fc1_b, fc2_w, fc2_b):
    f32 = np.float32
    fp8 = mybir.dt.np(FP8)
    emb_aug = np.zeros((V, EP), f32)
    emb_aug[:, :E] = np.asarray(emb, f32)
    emb_aug[0, :E] = 0.0
    emb_aug[:, E] = 1.0

    perm = np.r_[0:512, 768:1024, 512:768]  # i,f,g,o -> i,f,o,g

    def wx(w_ih, b_ih, b_hh):
        m = np.zeros((EP, G4H), f32)
        m[:E, :] = np.asarray(w_ih, f32).T
        m[E, :] = np.asarray(b_ih, f32) + np.asarray(b_hh, f32)
        m = m[:, perm]
        m[:, 768:] *= 2.0          # tanh(g) = 2*sigmoid(2g) - 1
        return m

    def wh(w_hh):
        m = np.asarray(w_hh, f32).T[:, perm].copy()
        m[:, 768:] *= 2.0
        return m

    wxf_full = wx(w_ih_f, b_ih_f, b_hh_f)
    wxb_full = wx(w_ih_b, b_ih_b, b_hh_b)
    fc1 = np.asarray(fc1_w, f32).T        # [512, 32]
    fc1_pk = np.concatenate(
        [_pack_dr(fc1[0:256]), _pack_dr(fc1[256:512])], axis=1)  # [128, 4*32]

    return dict(
        emb_aug=emb_aug.astype(bfloat16).copy(),
        wxf=_pack_dr(wxf_full[0:256]).astype(fp8).copy(),
        wxb=_pack_dr(wxb_full[0:256]).astype(fp8).copy(),
        wxf_t=wxf_full[256:304].astype(fp8).copy(),
        wxb_t=wxb_full[256:304].astype(fp8).copy(),
        whf=_pack_dr(wh(w_hh_f)).astype(fp8).copy(),
        whb=_pack_dr(wh(w_hh_b)).astype(fp8).copy(),
        fc1w=fc1_pk.astype(fp8).copy(),
        fc1b=np.asarray(fc1_b, f32).reshape(32, 1).copy(),
        fc2w=np.asarray(fc2_w, f32).T.astype(bfloat16).copy(),
        fc2b=np.asarray(fc2_b, f32).reshape(K, 1).copy(),
        iden=np.eye(128, dtype=f32).astype(bfloat16).copy(),
        iden8=np.eye(128, dtype=f32).astype(fp8).copy(),
    )


def _crf_host(emis, tags, mask, start_trans, trans, end_trans):
    # emis: [T, B, K] f32; exact forward algorithm in float64 on host
    emis = emis.astype(np.float64)
    trans = np.asarray(trans, np.float64)
    start = np.asarray(start_trans, np.float64)
    end = np.asarray(end_trans, np.float64)
    tags = np.asarray(tags, np.int64)
    m = np.asarray(mask, np.float64).T           # [T, B]
    tg = tags.T                                  # [T, B]
    Bsz = emis.shape[1]
    bidx = np.arange(Bsz)

    score = start[tg[0]] + emis[0, bidx, tg[0]]
    for t in range(1, emis.shape[0]):
        score = score + (trans[tg[t - 1], tg[t]] + emis[t, bidx, tg[t]]) * m[t]
    seq_ends = np.asarray(mask, np.int64).sum(1) - 1
    score = score + end[tg[seq_ends, bidx]]

    alpha = start[None, :] + emis[0]
    for t in range(1, emis.shape[0]):
        nxt = alpha[:, :, None] + trans[None] + emis[t][:, None, :]
        mx = nxt.max(axis=1)
        nxt = mx + np.log(np.exp(nxt - mx[:, None, :]).sum(axis=1))
        alpha = np.where(m[t][:, None] > 0, nxt, alpha)
    av = alpha + end[None, :]
    mx = av.max(axis=1)
    logZ = mx + np.log(np.exp(av - mx[:, None]).sum(axis=1))
    return -(score - logZ).mean()


_CACHE = {}


def _make_runner():
    import jax
    from jax.sharding import Mesh, PartitionSpec, NamedSharding
    try:
        from jax.experimental.shard_map import shard_map
    except ImportError:
        from jax import shard_map
    from concourse import bass2jax
    from concourse.bass2jax import _bass_exec_p, partition_id_tensor

    nc = build_bass(**BUILD_KWARGS)
    bass2jax.install_neuronx_cc_hook()
    partition_name = (nc.partition_id_tensor.name
                      if nc.partition_id_tensor else None)
    in_names, out_names, out_avals, zero_outs = [], [], [], []
    for alloc in nc.m.functions[0].allocations:
        if not isinstance(alloc, mybir.MemoryLocationSet):
            continue
        name = alloc.memorylocations[0].name
        if alloc.kind == "ExternalInput":
            if name != partition_name:
                in_names.append(name)
        elif alloc.kind == "ExternalOutput":
            shape = tuple(alloc.tensor_shape)
            dtype = mybir.dt.np(alloc.dtype)
            out_names.append(name)
            out_avals.append(jax.core.ShapedArray(shape, dtype))
            zero_outs.append(np.zeros(shape, dtype))
    n_params = len(in_names)
    in_names_all = in_names + out_names
    if partition_name is not None:
        in_names_all.append(partition_name)

    def _body(*args):
        operands = list(args)
        if partition_name is not None:
            operands.append(partition_id_tensor())
        return tuple(_bass_exec_p.bind(
            *operands, out_avals=tuple(out_avals),
            in_names=tuple(in_names_all), out_names=tuple(out_names),
            lowering_input_output_aliases=(),
            sim_require_finite=True, sim_require_nnan=True, nc=nc))

    devices = jax.devices()[:NCORES]
    mesh = Mesh(np.asarray(devices), ("core",))
    sh = NamedSharding(mesh, PartitionSpec("core"))
    # The kernel writes every element of its outputs, so the zero output
    # buffers are NOT donated: they are uploaded once and reused by every
    # execution (saves one ~70ms host->device sync per run).
    sm = shard_map(_body, mesh=mesh,
                   in_specs=(PartitionSpec("core"),) * (n_params + len(out_names)),
                   out_specs=(PartitionSpec("core"),) * len(out_names),
                   check_rep=False)
    return dict(jax=jax, sm=sm, sh=sh, in_names=in_names,
                out_names=out_names, zero_outs=zero_outs)


def _run_device(in_maps):
    if "rt" not in _CACHE:
        _CACHE["rt"] = _make_runner()
    rt = _CACHE["rt"]
    jax = rt["jax"]
    from concourse.bass2jax import fast_dispatch_compile
    concat_in = [np.concatenate([np.asarray(m[n]) for m in in_maps], 0)
                 for n in rt["in_names"]]
    rt["dev_in"] = [jax.device_put(a, rt["sh"]) for a in concat_in]
    rt["zo_dev"] = [jax.device_put(np.concatenate([z] * NCORES, 0), rt["sh"])
                    for z in rt["zero_outs"]]
    if "sharded" not in rt:
        args = tuple(rt["dev_in"]) + tuple(rt["zo_dev"])
        try:
            rt["sharded"] = fast_dispatch_compile(
                lambda: jax.jit(rt["sm"], keep_unused=True)
                .lower(*args).compile())
        except Exception:
            rt["sharded"] = jax.jit(rt["sm"], keep_unused=True)
    return _exec(rt)


def _exec(rt):
    outs = rt["sharded"](*rt["dev_in"], *rt["zo_dev"])
    e = np.asarray(outs[0])            # [NCORES*K, TOK]; blocks until done
    return [e[c * K:(c + 1) * K] for c in range(NCORES)]


def kernel_rerun(n=1):
    """Execute the compiled kernel n times back-to-back (one sync at the
    end) and return the last run's per-core outputs."""
    rt = _CACHE["rt"]
    outs = None
    for _ in range(n):
        outs = rt["sharded"](*rt["dev_in"], *rt["zo_dev"])
    e = np.asarray(outs[0])
    return [e[c * K:(c + 1) * K] for c in range(NCORES)]


def kernel(emb, w_ih_f, w_hh_f, b_ih_f, b_hh_f, w_ih_b, w_hh_b, b_ih_b,
           b_hh_b, fc1_w, fc1_b, fc2_w, fc2_b, start_trans, trans, end_trans,
           tokens, tags, mask):
    shared = _prep_shared(emb, w_ih_f, w_hh_f, b_ih_f, b_hh_f, w_ih_b,
                          w_hh_b, b_ih_b, b_hh_b, fc1_w, fc1_b, fc2_w, fc2_b)
    tokens = np.asarray(tokens)
    in_maps = []
    for c in range(NCORES):
        tk = tokens[c * BC:(c + 1) * BC, :].astype(np.int32)  # [BC, T]
        tk = tk.T.reshape(T * BC, 1).copy()                   # t-major
        in_maps.append({**shared, "toks": tk})

    core_emis = _run_device(in_maps)

    emis = np.zeros((T, B, K), np.float32)
    for c in range(NCORES):
        e = np.asarray(core_emis[c])                          # [K, T*BC]
        emis[:, c * BC:(c + 1) * BC, :] = (
            e.reshape(K, T, BC).transpose(1, 2, 0))
    loss = _crf_host(emis, tags, mask, start_trans, trans, end_trans)
    return np.float32(loss)


# revision 25
# speedup vs baseline: 2.4481x; 1.0740x over previous
"""BiLSTM-CRF forward loss on 8 Trainium2 NeuronCores.

Data-parallel: batch 64 -> 8 sequences per core. Each core runs
embedding gather -> BiLSTM(T=512,H=256) -> fc1(32)+relu -> fc2(4),
and outputs its emissions [4, T*8]. The tiny CRF dynamic program
(O(T*B*K^2), K=4) and the final mean run on host in numpy.

v2: FP8 DoubleRow matmuls (256-deep contraction per instruction) for
the recurrent, input and fc1 weights; the xg add is folded into PSUM
accumulation via an fp8 identity matmul; tanh(g) is computed as
2*sigmoid(2g)-1 with the 2x folded into the weights and the affine
fixup fused into the DVE c-update, so one sigmoid activation covers
all 64 gate columns; the h-gate multiply runs on the Pool engine.
"""

import sys
for _p in ("/opt/trn_rl_repo", "/root/.axon_site/_ro/trn_rl_repo"):
    if _p not in sys.path:
        sys.path.insert(0, _p)
import numpy as np
from ml_dtypes import bfloat16

import concourse.bass as bass
import concourse.bacc as bacc
import concourse.mybir as mybir
from concourse.tile import TileContext
from concourse import bass_utils

B, T, E, H, V, K = 64, 512, 300, 256, 50000, 4
NCORES = 8
BC = B // NCORES          # 8 sequences per core
EP = 304                  # E padded to 304; row 300 = ones (bias trick)
G4H = 4 * H               # 1024
F32 = mybir.dt.float32
BF16 = mybir.dt.bfloat16
I32 = mybir.dt.int32
FP8 = mybir.dt.float8e4
AF = mybir.ActivationFunctionType
ALU = mybir.AluOpType
DR = mybir.MatmulPerfMode.DoubleRow


BUILD_KWARGS = {}


def build_bass(t_steps=T, bc=BC, parts="012f", dr=True, dr2=False,
               rec_steps=None, elem=True):
    TOK = t_steps * bc
    nc = bacc.Bacc()

    # ---- DRAM parameters ----
    emb_aug = nc.dram_tensor("emb_aug", [V, EP], BF16, kind="ExternalInput")
    toks = nc.dram_tensor("toks", [TOK, 1], I32, kind="ExternalInput")
    # DoubleRow-packed input weights: rows 0:256 -> [128, 2*G4H], tail 48
    wxf = nc.dram_tensor("wxf", [128, 2 * G4H], FP8, kind="ExternalInput")
    wxb = nc.dram_tensor("wxb", [128, 2 * G4H], FP8, kind="ExternalInput")
    wxf_t = nc.dram_tensor("wxf_t", [48, G4H], FP8, kind="ExternalInput")
    wxb_t = nc.dram_tensor("wxb_t", [48, G4H], FP8, kind="ExternalInput")
    whf = nc.dram_tensor("whf", [128, 2 * G4H], FP8, kind="ExternalInput")
    whb = nc.dram_tensor("whb", [128, 2 * G4H], FP8, kind="ExternalInput")
    fc1w = nc.dram_tensor("fc1w", [128, 4 * 32], FP8, kind="ExternalInput")
    fc1b = nc.dram_tensor("fc1b", [32, 1], F32, kind="ExternalInput")
    fc2w = nc.dram_tensor("fc2w", [32, K], BF16, kind="ExternalInput")
    fc2b = nc.dram_tensor("fc2b", [K, 1], F32, kind="ExternalInput")
    iden = nc.dram_tensor("iden", [128, 128], BF16, kind="ExternalInput")
    iden8 = nc.dram_tensor("iden8", [128, 128], FP8, kind="ExternalInput")
    out = nc.dram_tensor("out", [K, TOK], F32, kind="ExternalOutput")

    n_ttile = TOK // 128          # token tiles of 128
    n_n512 = TOK // 512           # 512-wide token chunks
    ek = [(0, 128), (128, 128), (256, 48)]   # E-chunks for transpose

    with TileContext(nc) as tc:
        with tc.tile_pool(name="const", bufs=1) as constp, \
             tc.tile_pool(name="persist", bufs=1) as pp:
            # constants in SBUF
            id_sb = constp.tile([128, 128], BF16, tag="iden")
            nc.sync.dma_start(id_sb[:], iden[:])
            id8_sb = constp.tile([128, 128], FP8, tag="iden8")
            nc.sync.dma_start(id8_sb[:], iden8[:])
            wx_sb, wxt_sb, wh_sb = {}, {}, {}
            for d, src, srct, srch in (("f", wxf, wxf_t, whf),
                                       ("b", wxb, wxb_t, whb)):
                w = constp.tile([128, 2 * G4H], FP8, tag=f"wx{d}")
                nc.sync.dma_start(w[:], src[:])
                wx_sb[d] = w
                w = constp.tile([48, G4H], FP8, tag=f"wxt{d}")
                nc.sync.dma_start(w[:], srct[:])
                wxt_sb[d] = w
                w = constp.tile([128, 2 * G4H], FP8, tag=f"wh{d}")
                nc.sync.dma_start(w[:], srch[:])
                wh_sb[d] = w
            fc1w_sb = constp.tile([128, 4 * 32], FP8, tag="fc1w")
            nc.sync.dma_start(fc1w_sb[:], fc1w[:])
            fc2w_sb = constp.tile([32, K], BF16, tag="fc2w")
            nc.sync.dma_start(fc2w_sb[:], fc2w[:])
            fc1b_sb = constp.tile([32, 1], F32, tag="fc1b")
            nc.sync.dma_start(fc1b_sb[:], fc1b[:])
            fc2b_sb = constp.tile([K, 1], F32, tag="fc2b")
            nc.sync.dma_start(fc2b_sb[:], fc2b[:])

            # persistent activations
            # xg lives in per-(dir,window) tiles xgn below; gate chunk order
            # (host permutes): i,i,f,f,o,o,g,g
            # h layout: [128, 2 hid-chunks * TOK] fp8, col = k*TOK + t*bc + b
            hT = {d: pp.tile([128, 2 * TOK], FP8, tag=f"h{d}", name=f"h{d}")
                  for d in "fb"}

            with tc.tile_pool(name="xt", bufs=1) as xtp, \
                 tc.tile_pool(name="xrp", bufs=8) as xrp, \
                 tc.tile_pool(name="rec", bufs=1) as recp, \
                 tc.tile_pool(name="st", bufs=3) as stp, \
                 tc.tile_pool(name="fc", bufs=1) as fcp, \
                 tc.tile_pool(name="emo", bufs=2) as emop, \
                 tc.tile_pool(name="ps0", bufs=2, space="PSUM") as ps0p, \
                 tc.tile_pool(name="pst", bufs=2, space="PSUM") as pstp, \
                 tc.tile_pool(name="ps2", bufs=2, space="PSUM") as ps2p:
                # ---------- phase 0+1, interleaved into phase 2 ------------
                # Token windows of WSTEP steps; xg for window w of dir d is
                # produced (gather -> transpose -> matmul) as PE filler work
                # during window w-1 of the recurrence, keeping the PE busy
                # (and its clock ramped) while the LSTM chain runs.
                WN = n_n512                     # windows (8)
                WSTEP = t_steps // WN           # steps per window (64)
                WTOK = WSTEP * bc               # tokens per window (512)
                TT = WTOK // 128                # token tiles per window (4)
                # xT01: k-tile pair (rows 0:128 | 128:256) for DoubleRow rhs
                xT01 = xtp.tile([128, 2 * TOK], FP8, tag="xT01", name="xT01")
                xT2 = xtp.tile([48, TOK], FP8, tag="xT2", name="xT2")
                idx_all = xtp.tile([128, n_ttile], I32, tag="idx_all")
                nc.gpsimd.dma_start(
                    idx_all[:],
                    toks[:].rearrange("(i p) one -> p (i one)", p=128))
                xT01v = xT01[:].rearrange("p (k t) -> p k t", k=2)
                wxv = {d: wx_sb[d][:].rearrange("p (k g) -> p k g", k=2)
                       for d in "fb"}
                xr_of = {}

                def emit_gather(i):
                    xr = xrp.tile([128, EP], BF16, tag="xr", name="xr")
                    xr_of[i] = xr
                    nc.gpsimd.indirect_dma_start(
                        out=xr[:], out_offset=None,
                        in_=emb_aug[:],
                        in_offset=bass.IndirectOffsetOnAxis(
                            ap=idx_all[:, i:i + 1], axis=0),
                    )

                def emit_transpose(i):
                    xr = xr_of.pop(i)
                    for ki, (r0, rn) in enumerate(ek):
                        pt = pstp.tile([128, 128], BF16, tag="tp")
                        nc.tensor.transpose(
                            out=pt[:rn, :], in_=xr[:, r0:r0 + rn],
                            identity=id_sb[:])
                        if ki < 2:
                            dst = xT01[:, ki * TOK + i * 128:
                                       ki * TOK + (i + 1) * 128]
                        else:
                            dst = xT2[:, i * 128:(i + 1) * 128]
                        nc.vector.tensor_copy(out=dst, in_=pt[:rn, :])

                xgn = {(d, n): pp.tile([128, 8 * WTOK], FP8, tag=f"xg{d}{n}",
                                       name=f"xg{d}{n}")
                       for d in "fb" for n in range(WN)}

                def emit_xg(d, n, mu):
                    ps = ps0p.tile([128, 512], F32, tag="mm")
                    if dr:
                        nc.tensor.matmul(
                            ps[:],
                            lhsT=wxv[d][:, :, mu * 128:(mu + 1) * 128],
                            rhs=xT01v[:, :, n * WTOK:(n + 1) * WTOK],
                            start=True, stop=False,
                            perf_mode=DR, skip_group_check=True)
                    else:
                        for ki in range(2):
                            nc.tensor.matmul(
                                ps[:],
                                lhsT=wxv[d][:, ki, mu * 128:(mu + 1) * 128],
                                rhs=xT01v[:, ki, n * WTOK:(n + 1) * WTOK],
                                start=(ki == 0), stop=False,
                                skip_group_check=True)
                    nc.tensor.matmul(
                        ps[:],
                        lhsT=wxt_sb[d][:, mu * 128:(mu + 1) * 128],
                        rhs=xT2[:, n * WTOK:(n + 1) * WTOK],
                        start=False, stop=True,
                        skip_group_check=True)
                    dst = xgn[(d, n)][:, mu * WTOK:(mu + 1) * WTOK]
                    if (mu + n) % 2 == 0:
                        nc.scalar.copy(out=dst, in_=ps[:])
                    else:
                        nc.vector.tensor_copy(out=dst, in_=ps[:])

                # --- prelude: first f window and last b window ---
                done_tr = set()
                pre_f = list(range(TT))
                pre_b = list(range(TT * (WN - 1), TT * WN))
                for i in pre_f + pre_b:
                    emit_gather(i)
                for i in pre_f + pre_b:
                    emit_transpose(i)
                    done_tr.add(i)
                for mu in range(8):
                    emit_xg("f", 0, mu)
                for mu in range(8):
                    emit_xg("b", WN - 1, mu)

                # --- filler schedule for windows 0..WN-2 ---
                go = []          # remaining gathers, need-order
                for w in range(1, WN):
                    for i in range(TT * w, TT * w + TT):
                        if i not in done_tr and i not in go:
                            go.append(i)
                    bb = WN - 1 - w
                    for i in range(TT * bb, TT * bb + TT):
                        if i not in done_tr and i not in go:
                            go.append(i)
                win_items = [[] for _ in range(WN)]
                win_items[0].extend(
                    (lambda i=i: emit_gather(i)) for i in go)
                for k in range(WN - 1):
                    fw, bw = k + 1, WN - 2 - k
                    for i in list(range(TT * fw, TT * fw + TT)) + \
                            list(range(TT * bw, TT * bw + TT)):
                        if i not in done_tr:
                            done_tr.add(i)
                            win_items[k].append(lambda i=i: emit_transpose(i))
                    for mu in range(8):
                        win_items[k].append(
                            lambda d="f", n=fw, mu=mu: emit_xg(d, n, mu))
                    for mu in range(8):
                        win_items[k].append(
                            lambda d="b", n=bw, mu=mu: emit_xg(d, n, mu))
                win_done = [0] * WN

                def drain_filler(step):
                    k = step // WSTEP
                    if k >= WN:
                        return
                    items = win_items[k]
                    target = ((step % WSTEP) + 1) * len(items)
                    target = -(-target // WSTEP)     # ceil
                    while win_done[k] < target:
                        items[win_done[k]]()
                        win_done[k] += 1

                # -------- phase 2: recurrence, fwd+bwd interleaved ----------
                # sa cols per dir: [i(16) f(16) o(16) g2(16)] (ki-major pairs)
                c_st = {d: recp.tile([128, 2 * bc], F32, tag=f"c{d}",
                                     name=f"c{d}") for d in "fb"}
                hTv = {d: hT[d][:].rearrange("p (k t) -> p k t", k=2)
                       for d in "fb"}
                whv = {d: wh_sb[d][:].rearrange("p (k g) -> p k g", k=2)
                       for d in "fb"}
                # Emission is rank-ordered across the two directions so the
                # in-order engine queues never head-of-line block one
                # direction's chain behind the other's: PE(f,b) -> sigma(f,b)
                # -> DVE c-update(f,b) -> tanh(f,b) -> h(f,b).
                for step in range(t_steps if rec_steps is None else rec_steps):
                    first = step == 0
                    tt, ps_d, sa_d, t2_d, tc_d = {}, {}, {}, {}, {}
                    for d in "fb":
                        t = step if d == "f" else t_steps - 1 - step
                        tt[d] = t
                        ps = ps2p.tile([128, 8 * bc], F32, tag=f"ps{d}",
                                       name=f"ps{d}")
                        ps_d[d] = ps
                        tw, wo = t // WSTEP, t % WSTEP
                        xga = xgn[(d, tw)][:].rearrange(
                            "p (m t) -> p m t", m=8)[:, :, wo * bc:(wo + 1) * bc]
                        # xg add via fp8 identity matmul (starts accumulation)
                        nc.tensor.matmul(
                            ps[:], lhsT=id8_sb[:], rhs=xga,
                            start=True, stop=first, skip_group_check=True)
                        if not first:
                            tprev = t - 1 if d == "f" else t + 1
                            for mu in range(8):
                                if dr2:
                                    nc.tensor.matmul(
                                        ps[:, mu * bc:(mu + 1) * bc],
                                        lhsT=whv[d][:, :, mu * 128:(mu + 1) * 128],
                                        rhs=hTv[d][:, :, tprev * bc:(tprev + 1) * bc],
                                        start=False, stop=True,
                                        perf_mode=DR, skip_group_check=True)
                                else:
                                    for ki in range(2):
                                        nc.tensor.matmul(
                                            ps[:, mu * bc:(mu + 1) * bc],
                                            lhsT=whv[d][:, ki, mu * 128:(mu + 1) * 128],
                                            rhs=hTv[d][:, ki, tprev * bc:(tprev + 1) * bc],
                                            start=False, stop=(ki == 1),
                                            skip_group_check=True)
                    if not elem:
                        # timing-probe mode: write hT straight from psum
                        for d in "fb":
                            nc.vector.tensor_copy(
                                out=hTv[d][:, :, tt[d] * bc:(tt[d] + 1) * bc],
                                in_=ps_d[d][:, 0:2 * bc].rearrange(
                                    "p (k c) -> p k c", k=2))
                        drain_filler(step)
                        continue
                    for d in "fb":
                        sa = stp.tile([128, 8 * bc], F32, tag=f"sa{d}",
                                      name=f"sa{d}")
                        sa_d[d] = sa
                        nc.scalar.activation(sa[:], ps_d[d][:], AF.Sigmoid)
                    for d in "fb":
                        # c = sig_f*c + sig_i*(2*sig_g2 - 1)
                        sa = sa_d[d]
                        t2 = stp.tile([128, 2 * bc], F32, tag=f"t2{d}",
                                      name=f"t2{d}")
                        t2_d[d] = t2
                        nc.vector.scalar_tensor_tensor(
                            out=t2[:], in0=sa[:, 6 * bc:8 * bc], scalar=-0.5,
                            in1=sa[:, 0:2 * bc], op0=ALU.add, op1=ALU.mult)
                        if first:
                            nc.vector.tensor_scalar_mul(
                                c_st[d][:], t2[:], 2.0)
                        else:
                            t1 = stp.tile([128, 2 * bc], F32, tag=f"t1{d}",
                                          name=f"t1{d}")
                            nc.vector.tensor_tensor(
                                out=t1[:], in0=sa[:, 2 * bc:4 * bc],
                                in1=c_st[d][:], op=ALU.mult)
                            nc.vector.scalar_tensor_tensor(
                                out=c_st[d][:], in0=t2[:], scalar=2.0,
                                in1=t1[:], op0=ALU.mult, op1=ALU.add)
                    for d in "fb":
                        tcl = stp.tile([128, 2 * bc], F32, tag=f"tc{d}",
                                       name=f"tc{d}")
                        tc_d[d] = tcl
                        nc.scalar.activation(tcl[:], c_st[d][:], AF.Tanh)
                    for d in "fb":
                        # h = sig_o * tanh(c), fp8 out
                        nc.vector.tensor_tensor(
                            out=hTv[d][:, :, tt[d] * bc:(tt[d] + 1) * bc],
                            in0=sa_d[d][:, 4 * bc:6 * bc].rearrange(
                                "p (k c) -> p k c", k=2),
                            in1=tc_d[d][:].rearrange("p (k c) -> p k c", k=2),
                            op=ALU.mult)
                    drain_filler(step)

                # leftovers (short rec_steps ablations)
                for k in range(WN):
                    while win_done[k] < len(win_items[k]):
                        win_items[k][win_done[k]]()
                        win_done[k] += 1

                # ---------- phase 3: fc1 + relu, fc2 + bias, out ----------
                z = fcp.tile([32, TOK], BF16, tag="z")
                fc1v = fc1w_sb[:].rearrange("p (dk c) -> p dk c", dk=4)
                n_n512_f = n_n512 if "f" in parts else 0
                for n in range(n_n512_f):
                    ps = ps0p.tile([32, 512], F32, tag="mm")
                    for di, d in enumerate("fb"):
                        if dr:
                            nc.tensor.matmul(
                                ps[:],
                                lhsT=fc1v[:, 2 * di:2 * di + 2, :],
                                rhs=hTv[d][:, :, n * 512:(n + 1) * 512],
                                start=(di == 0), stop=(di == 1),
                                perf_mode=DR, skip_group_check=True)
                        else:
                            for ki in range(2):
                                nc.tensor.matmul(
                                    ps[:],
                                    lhsT=fc1v[:, 2 * di + ki, :],
                                    rhs=hTv[d][:, ki, n * 512:(n + 1) * 512],
                                    start=(di == 0 and ki == 0),
                                    stop=(di == 1 and ki == 1),
                                    skip_group_check=True)
                    nc.scalar.activation(z[:, n * 512:(n + 1) * 512], ps[:],
                                         AF.Relu, bias=fc1b_sb[:, :1])
                for n in range(n_n512_f):
                    ps = ps0p.tile([K, 512], F32, tag="mm")
                    nc.tensor.matmul(ps[:], lhsT=fc2w_sb[:],
                                     rhs=z[:, n * 512:(n + 1) * 512],
                                     start=True, stop=True)
                    em = emop.tile([K, 512], F32, tag="em", name="em")
                    nc.vector.tensor_scalar_add(em[:], ps[:], fc2b_sb[:, :1])
                    nc.sync.dma_start(out[:, n * 512:(n + 1) * 512], em[:])
    nc.compile()
    return nc


def _pack_dr(m):
    """[256, N] -> DoubleRow-packed [128, 2*N]: out[p, k*N+j] = m[k*128+p, j]"""
    n = m.shape[1]
    return m.reshape(2, 128, n).transpose(1, 0, 2).reshape(128, 2 * n)


def _prep_shared(emb, w_ih_f, w_hh_f, b_ih_f, b_hh_f, w_ih_b, w_hh_b,
                 b_ih_b, b_hh_b, fc1_w, fc1_b, fc2_w, fc2_b):
    f32 = np.float32
    fp8 = mybir.dt.np(FP8)
    emb_aug = np.zeros((V, EP), f32)
    emb_aug[:, :E] = np.asarray(emb, f32)
    emb_aug[0, :E] = 0.0
    emb_aug[:, E] = 1.0

    perm = np.r_[0:512, 768:1024, 512:768]  # i,f,g,o -> i,f,o,g

    def wx(w_ih, b_ih, b_hh):
        m = np.zeros((EP, G4H), f32)
        m[:E, :] = np.asarray(w_ih, f32).T
        m[E, :] = np.asarray(b_ih, f32) + np.asarray(b_hh, f32)
        m = m[:, perm]
        m[:, 768:] *= 2.0          # tanh(g) = 2*sigmoid(2g) - 1
        return m

    def wh(w_hh):
        m = np.asarray(w_hh, f32).T[:, perm].copy()
        m[:, 768:] *= 2.0
        return m

    wxf_full = wx(w_ih_f, b_ih_f, b_hh_f)
    wxb_full = wx(w_ih_b, b_ih_b, b_hh_b)
    fc1 = np.asarray(fc1_w, f32).T        # [512, 32]
    fc1_pk = np.concatenate(
        [_pack_dr(fc1[0:256]), _pack_dr(fc1[256:512])], axis=1)  # [128, 4*32]

    return dict(
        emb_aug=emb_aug.astype(bfloat16).copy(),
        wxf=_pack_dr(wxf_full[0:256]).astype(fp8).copy(),
        wxb=_pack_dr(wxb_full[0:256]).astype(fp8).copy(),
        wxf_t=wxf_full[256:304].astype(fp8).copy(),
        wxb_t=wxb_full[256:304].astype(fp8).copy(),
        whf=_pack_dr(wh(w_hh_f)).astype(fp8).copy(),
        whb=_pack_dr(wh(w_hh_b)).astype(fp8).copy(),
        fc1w=fc1_pk.astype(fp8).copy(),
        fc1b=np.asarray(fc1_b, f32).reshape(32, 1).copy(),
        fc2w=np.asarray(fc2_w, f32).T.astype(bfloat16).copy(),
        fc2b=np.asarray(fc2_b, f32).reshape(K, 1).copy(),
        iden=np.eye(128, dtype=f32).astype(bfloat16).copy(),
        iden8=np.eye(128, dtype=f32).astype(fp8).copy(),
    )


def _crf_host(emis, tags, mask, start_trans, trans, end_trans):
    # emis: [T, B, K] f32; exact forward algorithm in float64 on host
    emis = emis.astype(np.float64)
    trans = np.asarray(trans, np.float64)
    start = np.asarray(start_trans, np.float64)
    end = np.asarray(end_trans, np.float64)
    tags = np.asarray(tags, np.int64)
    m = np.asarray(mask, np.float64).T           # [T, B]
    tg = tags.T                                  # [T, B]
    Bsz = emis.shape[1]
    bidx = np.arange(Bsz)

    score = start[tg[0]] + emis[0, bidx, tg[0]]
    for t in range(1, emis.shape[0]):
        score = score + (trans[tg[t - 1], tg[t]] + emis[t, bidx, tg[t]]) * m[t]
    seq_ends = np.asarray(mask, np.int64).sum(1) - 1
    score = score + end[tg[seq_ends, bidx]]

    alpha = start[None, :] + emis[0]
    for t in range(1, emis.shape[0]):
        nxt = alpha[:, :, None] + trans[None] + emis[t][:, None, :]
        mx = nxt.max(axis=1)
        nxt = mx + np.log(np.exp(nxt - mx[:, None, :]).sum(axis=1))
        alpha = np.where(m[t][:, None] > 0, nxt, alpha)
    av = alpha + end[None, :]
    mx = av.max(axis=1)
    logZ = mx + np.log(np.exp(av - mx[:, None]).sum(axis=1))
    return -(score - logZ).mean()


_CACHE = {}


def _make_runner():
    import jax
    from jax.sharding import Mesh, PartitionSpec, NamedSharding
    try:
        from jax.experimental.shard_map import shard_map
    except ImportError:
        from jax import shard_map
    from concourse import bass2jax
    from concourse.bass2jax import _bass_exec_p, partition_id_tensor

    nc = build_bass(**BUILD_KWARGS)
    bass2jax.install_neuronx_cc_hook()
    partition_name = (nc.partition_id_tensor.name
                      if nc.partition_id_tensor else None)
    in_names, out_names, out_avals, zero_outs = [], [], [], []
    for alloc in nc.m.functions[0].allocations:
        if not isinstance(alloc, mybir.MemoryLocationSet):
            continue
        name = alloc.memorylocations[0].name
        if alloc.kind == "ExternalInput":
            if name != partition_name:
                in_names.append(name)
        elif alloc.kind == "ExternalOutput":
            shape = tuple(alloc.tensor_shape)
            dtype = mybir.dt.np(alloc.dtype)
            out_names.append(name)
            out_avals.append(jax.core.ShapedArray(shape, dtype))
            zero_outs.append(np.zeros(shape, dtype))
    n_params = len(in_names)
    in_names_all = in_names + out_names
    if partition_name is not None:
        in_names_all.append(partition_name)

    def _body(*args):
        operands = list(args)
        if partition_name is not None:
            operands.append(partition_id_tensor())
        return tuple(_bass_exec_p.bind(
            *operands, out_avals=tuple(out_avals),
            in_names=tuple(in_names_all), out_names=tuple(out_names),
            lowering_input_output_aliases=(),
            sim_require_finite=True, sim_require_nnan=True, nc=nc))

    devices = jax.devices()[:NCORES]
    mesh = Mesh(np.asarray(devices), ("core",))
    sh = NamedSharding(mesh, PartitionSpec("core"))
    # The kernel writes every element of its outputs, so the zero output
    # buffers are NOT donated: they are uploaded once and reused by every
    # execution (saves one ~70ms host->device sync per run).
    sm = shard_map(_body, mesh=mesh,
                   in_specs=(PartitionSpec("core"),) * (n_params + len(out_names)),
                   out_specs=(PartitionSpec("core"),) * len(out_names),
                   check_rep=False)
    return dict(jax=jax, sm=sm, sh=sh, in_names=in_names,
                out_names=out_names, zero_outs=zero_outs)


def _run_device(in_maps):
    if "rt" not in _CACHE:
        _CACHE["rt"] = _make_runner()
    rt = _CACHE["rt"]
    jax = rt["jax"]
    from concourse.bass2jax import fast_dispatch_compile
    concat_in = [np.concatenate([np.asarray(m[n]) for m in in_maps], 0)
                 for n in rt["in_names"]]
    rt["dev_in"] = [jax.device_put(a, rt["sh"]) for a in concat_in]
    rt["zo_dev"] = [jax.device_put(np.concatenate([z] * NCORES, 0), rt["sh"])
                    for z in rt["zero_outs"]]
    if "sharded" not in rt:
        args = tuple(rt["dev_in"]) + tuple(rt["zo_dev"])
        try:
            rt["sharded"] = fast_dispatch_compile(
                lambda: jax.jit(rt["sm"], keep_unused=True)
                .lower(*args).compile())
        except Exception:
            rt["sharded"] = jax.jit(rt["sm"], keep_unused=True)
    return _exec(rt)


def _exec(rt):
    outs = rt["sharded"](*rt["dev_in"], *rt["zo_dev"])
    e = np.asarray(outs[0])            # [NCORES*K, TOK]; blocks until done
    return [e[c * K:(c + 1) * K] for c in range(NCORES)]


def kernel_rerun(n=1):
    """Execute the compiled kernel n times back-to-back (one sync at the
    end) and return the last run's per-core outputs."""
    rt = _CACHE["rt"]
    outs = None
    for _ in range(n):
        outs = rt["sharded"](*rt["dev_in"], *rt["zo_dev"])
    e = np.asarray(outs[0])
    return [e[c * K:(c + 1) * K] for c in range(NCORES)]


def kernel(emb, w_ih_f, w_hh_f, b_ih_f, b_hh_f, w_ih_b, w_hh_b, b_ih_b,
           b_hh_b, fc1_w, fc1_b, fc2_w, fc2_b, start_trans, trans, end_trans,
           tokens, tags, mask):
    shared = _prep_shared(emb, w_ih_f, w_hh_f, b_ih_f, b_hh_f, w_ih_b,
                          w_hh_b, b_ih_b, b_hh_b, fc1_w, fc1_b, fc2_w, fc2_b)
    tokens = np.asarray(tokens)
    in_maps = []
    for c in range(NCORES):
        tk = tokens[c * BC:(c + 1) * BC, :].astype(np.int32)  # [BC, T]
        tk = tk.T.reshape(T * BC, 1).copy()                   # t-major
        in_maps.append({**shared, "toks": tk})

    core_emis = _run_device(in_maps)

    emis = np.zeros((T, B, K), np.float32)
    for c in range(NCORES):
        e = np.asarray(core_emis[c])                          # [K, T*BC]
        emis[:, c * BC:(c + 1) * BC, :] = (
            e.reshape(K, T, BC).transpose(1, 2, 0))
    loss = _crf_host(emis, tags, mask, start_trans, trans, end_trans)
    return np.float32(loss)
